# revision 1
# baseline (speedup 1.0000x reference)
"""CriticSwapGNN Trainium2 kernel: 4-layer GAT + MLP head + graph mean pool.

Sharding: nodes in 128-blocks, 8 cores x 49 blocks (dst-range ownership).
Edges sorted by dst, per dst-block, split lo/hi by src half (int16 gather
indices), tiled 128/tile. Per layer-launch: edge phase (dma_gather of xp rows,
on-chip segment softmax via one-hot matmuls) + node phase (xp_next = x_next@W).
Host concatenates per-core xp slices between launches.
"""
import os
import sys
import time
import numpy as np

if '/opt/trn_rl_repo' not in sys.path:
    sys.path.insert(0, '/opt/trn_rl_repo')

N = 50000; E = 800000; F = 16; HID = 128; H = 4; C = 32; FC = 256; NL = 15; NG = 8
NCORES = 8
BLK = 128
BPC = 49                      # blocks per core (uniform; core 7 pads)
NPAD = NCORES * BPC * BLK     # 50176
HALF = 4 * BPC * BLK          # 25088 (cores 0-3 own lo half)
CHUNK_BLKS = 2

_cache = {}


def _build_host(inputs):
    import concourse.mybir as mybir  # noqa: F401  (path check)
    src = np.asarray(inputs['edge_index'][0], np.int64)
    dst = np.asarray(inputs['edge_index'][1], np.int64)
    lat = np.asarray(inputs['latency'], np.float32)

    # ---- per (core, block) edge lists, sorted by dst ----
    order = np.argsort(dst, kind='stable')
    es, ed, el = src[order], dst[order], lat[order]
    blk_of = ed // BLK
    blk_starts = np.searchsorted(blk_of, np.arange(NCORES * BPC + 1))
    per = {}
    tlo = np.zeros((NCORES, BPC), np.int64)
    thi = np.zeros((NCORES, BPC), np.int64)
    for k in range(NCORES):
        for b in range(BPC):
            g = k * BPC + b
            s_, e_ = blk_starts[g], blk_starts[g + 1]
            bs, bd, bl = es[s_:e_], ed[s_:e_] - g * BLK, el[s_:e_]
            lo = bs < HALF
            per[(k, b)] = (bs[lo], bd[lo], bl[lo], bs[~lo] - HALF, bd[~lo], bl[~lo])
            tlo[k, b] = -(-len(bs[lo]) // 128)
            thi[k, b] = -(-len(bs[~lo]) // 128)
    TLO = tlo.max(axis=0)     # uniform tile layout across cores
    THI = thi.max(axis=0)

    # chunk layout: blocks grouped CHUNK_BLKS at a time
    chunks = []
    b = 0
    while b < BPC:
        blks = list(range(b, min(b + CHUNK_BLKS, BPC)))
        chunks.append(blks)
        b += CHUNK_BLKS
    ntiles = int((TLO + THI).sum())

    # ---- pack per-core arrays in the uniform layout ----
    idx_lo = np.zeros((NCORES, 128, int(TLO.sum()) * 8), np.int16)
    idx_hi = np.zeros((NCORES, 128, int(THI.sum()) * 8), np.int16)
    dstcol = np.full((NCORES, 128, ntiles), BLK, np.float32)   # pad col -> 128
    latcol = np.zeros((NCORES, 128, ntiles), np.float32)
    lo_off = np.concatenate([[0], np.cumsum(TLO)])
    hi_off = np.concatenate([[0], np.cumsum(THI)])

    def wrap16(a):
        return np.tile(a.astype(np.int16).reshape(-1, 16).T, (8, 1))

    # tile order within the global tile axis: block-major, lo tiles then hi
    tile_pos = []
    for b in range(BPC):
        for t in range(int(TLO[b])):
            tile_pos.append(('lo', b, t))
        for t in range(int(THI[b])):
            tile_pos.append(('hi', b, t))
    tp_index = {v: i for i, v in enumerate(tile_pos)}

    for k in range(NCORES):
        for b in range(BPC):
            slo, dlo, llo, shi, dhi, lhi = per[(k, b)]
            for half, s_, d_, l_, T_, off in (
                    ('lo', slo, dlo, llo, TLO, lo_off), ('hi', shi, dhi, lhi, THI, hi_off)):
                nt = int(T_[b])
                cap = nt * 128
                sp = np.zeros(cap, np.int64)
                dp = np.full(cap, BLK, np.int64)
                lp = np.zeros(cap, np.float32)
                sp[:len(s_)] = s_
                dp[:len(d_)] = d_
                lp[:len(l_)] = l_
                if nt:
                    w = wrap16(sp)
                    if half == 'lo':
                        idx_lo[k][:, int(off[b]) * 8:(int(off[b]) + nt) * 8] = w
                    else:
                        idx_hi[k][:, int(off[b]) * 8:(int(off[b]) + nt) * 8] = w
                    for t in range(nt):
                        gi = tp_index[(half, b, t)]
                        dstcol[k][:, gi] = dp[t * 128:(t + 1) * 128]
                        latcol[k][:, gi] = lp[t * 128:(t + 1) * 128]

    # ---- features / weights folding (host: index prep + weight folding only) ----
    type_ids = np.asarray(inputs['type_ids'], np.int64)
    onehot4T = np.zeros((NCORES, 4, BPC * BLK), np.float32)
    for k in range(NCORES):
        sl = slice(k * BPC * BLK, (k + 1) * BPC * BLK)
        ids = np.full(BPC * BLK, -1, np.int64)
        n_real = max(0, min(N - k * BPC * BLK, BPC * BLK))
        ids[:n_real] = type_ids[k * BPC * BLK:k * BPC * BLK + n_real]
        for t in range(4):
            onehot4T[k, t] = (ids == t).astype(np.float32)

    def wrapnode(x):  # [N] -> [128, 392] node-major blocks, zero pad
        o = np.zeros(NPAD, np.float32)
        o[:N] = x
        return o.reshape(-1, 128).T.copy()   # node n=128b+p -> [p, b]

    req_w_full = wrapnode(np.asarray(inputs['requests'], np.float32))
    us_w_full = wrapnode(np.asarray(inputs['update_step'], np.float32))
    idx_node = np.arange(NPAD).reshape(-1, 128).T
    mask_ge15 = ((idx_node >= NL) & (idx_node < N)).astype(np.float32)
    mask_lt15 = (idx_node < NL).astype(np.float32)

    # per-core column perm: own blocks first
    perms = []
    for k in range(NCORES):
        own = np.arange(k * BPC, (k + 1) * BPC)
        rest = np.array([c for c in range(NPAD // 128) if c not in set(own)])
        perms.append(np.concatenate([own, rest]))

    def a_rep(a):  # [H,C] -> [128, HID] replicated rows
        return np.tile(a.reshape(1, HID).astype(np.float32), (128, 1))

    def we_fold(We, a_e):
        We = np.asarray(We, np.float32); a_e = np.asarray(a_e, np.float32)
        return np.array([(We[0, h * C:(h + 1) * C] * a_e[h]).sum() for h in range(H)], np.float32)

    W0 = np.asarray(inputs['W0'], np.float32)
    T0 = (np.asarray(inputs['emb'], np.float32) @ W0[:F]).astype(np.float32)
    layers = []
    layers.append(dict(a_s=a_rep(np.asarray(inputs['as0'])), a_d=a_rep(np.asarray(inputs['ad0'])),
                       we=we_fold(inputs['We0'], inputs['ae0']), b=np.asarray(inputs['b0'], np.float32),
                       Wn=np.asarray(inputs['Wh'][0], np.float32), relu=True))
    layers.append(dict(a_s=a_rep(np.asarray(inputs['ash'][0])), a_d=a_rep(np.asarray(inputs['adh'][0])),
                       we=we_fold(np.asarray(inputs['Weh'][0]).reshape(1, -1), inputs['aeh'][0]),
                       b=np.asarray(inputs['bh'][0], np.float32),
                       Wn=np.asarray(inputs['Wh'][1], np.float32), relu=True))
    layers.append(dict(a_s=a_rep(np.asarray(inputs['ash'][1])), a_d=a_rep(np.asarray(inputs['adh'][1])),
                       we=we_fold(np.asarray(inputs['Weh'][1]).reshape(1, -1), inputs['aeh'][1]),
                       b=np.asarray(inputs['bh'][1], np.float32),
                       Wn=np.asarray(inputs['Wf'], np.float32), relu=True))
    layers.append(dict(a_s=a_rep(np.asarray(inputs['asf'])), a_d=a_rep(np.asarray(inputs['adf'])),
                       we=we_fold(inputs['Wef'], inputs['aef']), b=np.asarray(inputs['bf'], np.float32),
                       Wn=np.eye(HID, dtype=np.float32), relu=False))

    batch = np.asarray(inputs['batch'], np.int64)
    pool_mat = np.zeros((NCORES, 128, BPC * NG), np.float32)
    cnt = np.zeros(NG, np.float64)
    np.add.at(cnt, batch, 1.0)
    for k in range(NCORES):
        for b in range(BPC):
            base = (k * BPC + b) * BLK
            for p in range(128):
                n_ = base + p
                if n_ < N:
                    pool_mat[k, p, b * NG + batch[n_]] = 1.0

    host = dict(
        TLO=TLO, THI=THI, chunks=chunks, ntiles=ntiles, lo_off=lo_off, hi_off=hi_off,
        tile_pos=tile_pos, idx_lo=idx_lo, idx_hi=idx_hi, dstcol=dstcol, latcol=latcol,
        onehot4T=onehot4T, req_w_full=req_w_full, us_w_full=us_w_full,
        mask_ge15=mask_ge15, mask_lt15=mask_lt15, perms=perms, T0=T0,
        w16_rep=np.tile(W0[F][None, :], (128, 1)).astype(np.float32),
        w17_rep=np.tile(W0[F + 1][None, :], (128, 1)).astype(np.float32),
        layers=layers, cnt=cnt, pool_mat=pool_mat,
        C1w=np.asarray(inputs['C1w'], np.float32), C1b=np.asarray(inputs['C1b'], np.float32),
        C2w=np.asarray(inputs['C2w'], np.float32), C2b=np.asarray(inputs['C2b'], np.float32),
        C3w=np.asarray(inputs['C3w'], np.float32), C3b=np.asarray(inputs['C3b'], np.float32),
        iota_row=np.tile(np.arange(128, dtype=np.float32)[None, :], (128, 1)),
        ident=np.eye(128, dtype=np.float32),
        ones_col=np.ones((128, 1), np.float32),
    )
    return host


# ---------------------------------------------------------------- programs
def _mk(name_shapes, nc, kind):
    out = {}
    import concourse.mybir as mybir
    for name, (shape, dt) in name_shapes.items():
        out[name] = nc.dram_tensor(name, list(shape), dt, kind=kind)
    return out


def _edge_phase(tc, c, host, relu, sdst, xslice, pools):
    """Edge phase: reads gather table (DRAM tensors c['tab']), writes xslice."""
    import concourse.mybir as mybir
    nc = tc.nc
    F32 = mybir.dt.float32
    ALU = mybir.AluOpType
    AX = mybir.AxisListType
    ACTF = mybir.ActivationFunctionType
    constp, gbufp, workp, chunkp, psA, psB = pools
    TLO, THI, lo_off, hi_off = host['TLO'], host['THI'], host['lo_off'], host['hi_off']
    tp_index = {v: i for i, v in enumerate(host['tile_pos'])}

    for blks in host['chunks']:
        glo = int(sum(TLO[b] for b in blks))
        ghi = int(sum(THI[b] for b in blks))
        Tch = glo + ghi
        g_lo = gbufp.tile([128, max(glo, 1), HID], F32, tag="g_lo")
        g_hi = gbufp.tile([128, max(ghi, 1), HID], F32, tag="g_hi")
        if glo:
            nc.gpsimd.dma_gather(g_lo[:, 0:glo, :], c['tab'][0:HALF, :],
                                 c['idx_lo'][:, int(lo_off[blks[0]]) * 8:(int(lo_off[blks[0]]) + glo) * 8],
                                 glo * 128, glo * 128, HID, single_packet=False)
        if ghi:
            nc.gpsimd.dma_gather(g_hi[:, 0:ghi, :], c['tab'][HALF:NPAD, :],
                                 c['idx_hi'][:, int(hi_off[blks[0]]) * 8:(int(hi_off[blks[0]]) + ghi) * 8],
                                 ghi * 128, ghi * 128, HID, single_packet=False)

        s_src = chunkp.tile([128, max(Tch, 1), H], F32, tag="s_src")
        s_dst_e = chunkp.tile([128, max(Tch, 1), H], F32, tag="s_dst_e")
        oh_ch = chunkp.tile([128, max(Tch, 1), 128], F32, tag="oh_ch")
        araw = chunkp.tile([128, max(Tch, 1), H], F32, tag="araw")
        wexp = chunkp.tile([128, max(Tch, 1), H], F32, tag="wexp")

        # chunk-local tile enumeration: (kind, block, gather slot, chunk slot)
        tl = []
        li = hi = 0
        for b in blks:
            for t in range(int(TLO[b])):
                tl.append(('lo', b, li, len(tl))); li += 1
            for t in range(int(THI[b])):
                tl.append(('hi', b, hi, len(tl))); hi += 1

        # pass A
        for half, b, g, t in tl:
            xg = (g_lo if half == 'lo' else g_hi)[:, g, :]
            gidx = int((lo_off[b] if half == 'lo' else hi_off[b]) + g - (lo_off[blks[0]] if half == 'lo' else hi_off[blks[0]])) if False else None
            # global tile index for dstcol/latcol
            ti = tp_index[(half, b, g - int((lo_off[b] - lo_off[blks[0]]) if half == 'lo' else (hi_off[b] - hi_off[blks[0]])))]
            xa = workp.tile([128, HID], F32, tag="xa")
            nc.vector.tensor_tensor(out=xa[:], in0=xg, in1=c['a_s_rep'][:], op=ALU.mult)
            nc.vector.tensor_reduce(out=s_src[:, t, :], in_=xa[:].rearrange("p (h c) -> p h c", h=H),
                                    op=ALU.add, axis=AX.X)
            oh = oh_ch[:, t, :]
            nc.vector.tensor_scalar(out=oh, in0=c['iota_row'][:], scalar1=c['dstcol'][:, ti:ti + 1],
                                    scalar2=None, op0=ALU.is_equal)
            tp = psA.tile([128, 128], F32, tag="tpsum")
            nc.tensor.transpose(tp[:], oh, c['ident'][:])
            ohT = workp.tile([128, 128], F32, tag="ohT")
            nc.scalar.copy(out=ohT[:], in_=tp[:])
            sp = psB.tile([128, H], F32, tag="spsum")
            nc.tensor.matmul(sp[:], ohT[:], sdst[:, b, :], start=True, stop=True)
            nc.scalar.copy(out=s_dst_e[:, t, :], in_=sp[:])

        # chunk araw pipeline
        lwslice = []
        for half, b, g, t in tl:
            ti = tp_index[(half, b, g - int((lo_off[b] - lo_off[blks[0]]) if half == 'lo' else (hi_off[b] - hi_off[blks[0]])))]
            lwslice.append(ti)
        latw = workp.tile([128, max(Tch, 1), H], F32, tag="latw")
        for j, ti in enumerate(lwslice):
            nc.vector.tensor_scalar(out=latw[:, j, :], in0=c['we_rep'][:],
                                    scalar1=c['latcol'][:, ti:ti + 1], scalar2=None, op0=ALU.mult)
        nc.vector.tensor_tensor(out=araw[:], in0=s_src[:], in1=s_dst_e[:], op=ALU.add)
        nc.vector.tensor_tensor(out=araw[:], in0=araw[:], in1=latw[:], op=ALU.add)
        lr = workp.tile([128, max(Tch, 1), H], F32, tag="lr")
        nc.vector.tensor_scalar(out=lr[:], in0=araw[:], scalar1=0.2, scalar2=None, op0=ALU.mult)
        nc.vector.tensor_tensor(out=araw[:], in0=araw[:], in1=lr[:], op=ALU.max)
        mx = workp.tile([128, H], F32, tag="mx")
        nc.vector.tensor_reduce(out=mx[:], in_=araw[:].rearrange("p t h -> p h t"), op=ALU.max, axis=AX.X)
        emx = workp.tile([128, H], F32, tag="emx")
        nc.scalar.activation(out=emx[:], in_=mx[:], func=ACTF.Exp)
        msum = psB.tile([1, H], F32, tag="small1")
        nc.tensor.matmul(msum[:], c['ones_col'][:], emx[:], start=True, stop=True)
        M_row = workp.tile([1, H], F32, tag="M_row")
        nc.scalar.activation(out=M_row[:], in_=msum[:], func=ACTF.Ln)
        M_rep = workp.tile([128, H], F32, tag="M_rep")
        nc.gpsimd.partition_broadcast(M_rep[:], M_row[:])
        nc.vector.tensor_tensor(out=araw[:], in0=araw[:],
                                in1=M_rep[:].rearrange("p h -> p () h").broadcast_to([128, max(Tch, 1), H]),
                                op=ALU.subtract)
        nc.scalar.activation(out=wexp[:], in_=araw[:], func=ACTF.Exp)

        # pass B per block
        for b in blks:
            bt = [v for v in tl if v[1] == b]
            dps = psB.tile([128, H], F32, tag="dpsum")
            ops = psB.tile([128, HID], F32, tag="opsum")
            for j, (half, _b, g, t) in enumerate(bt):
                xg = (g_lo if half == 'lo' else g_hi)[:, g, :]
                oh = oh_ch[:, t, :]
                nc.tensor.matmul(dps[:], oh, wexp[:, t, :], start=(j == 0), stop=(j == len(bt) - 1))
                wmsg = workp.tile([128, HID], F32, tag="wmsg")
                for h in range(H):
                    nc.scalar.activation(out=wmsg[:, h * C:(h + 1) * C], in_=xg[:, h * C:(h + 1) * C],
                                         func=ACTF.Copy, scale=wexp[:, t, h:h + 1])
                nc.tensor.matmul(ops[:], oh, wmsg[:], start=(j == 0), stop=(j == len(bt) - 1))
            den = workp.tile([128, H], F32, tag="den")
            nc.vector.tensor_scalar(out=den[:], in0=dps[:], scalar1=1e-16, scalar2=None, op0=ALU.add)
            recip = workp.tile([128, H], F32, tag="recip")
            nc.vector.reciprocal(out=recip[:], in_=den[:])
            xn = workp.tile([128, HID], F32, tag="xn")
            nc.vector.tensor_tensor(out=xn[:], in0=ops[:],
                                    in1=recip[:].rearrange("p h -> p h ()").broadcast_to([128, H, C]),
                                    op=ALU.mult)
            nc.vector.tensor_tensor(out=xn[:], in0=xn[:], in1=c['b_rep'][:], op=ALU.add)
            if relu:
                nc.scalar.activation(out=xslice[:, b, :], in_=xn[:], func=ACTF.Relu)
            else:
                nc.scalar.copy(out=xslice[:, b, :], in_=xn[:])


def _build_gat(host, mlp):
    """One GAT layer launch. mlp=False: node phase -> xp_next slice out.
    mlp=True: final layer + MLP + pool -> partials out."""
    import concourse.bacc as bacc
    import concourse.mybir as mybir
    import concourse.tile as tile
    from concourse import library_config
    F32 = mybir.dt.float32
    I16 = mybir.dt.int16
    ALU = mybir.AluOpType
    AX = mybir.AxisListType
    ACTF = mybir.ActivationFunctionType
    nc = bacc.Bacc("TRN2", target_bir_lowering=False, debug=False, num_devices=NCORES)

    nlo8, nhi8 = host['idx_lo'].shape[2], host['idx_hi'].shape[2]
    ntiles = host['ntiles']
    ins = {
        'tab': ([NPAD, HID], F32), 'xp_own': ([BPC * BLK, HID], F32),
        'idx_lo': ([128, nlo8], I16), 'idx_hi': ([128, nhi8], I16),
        'dstcol': ([128, ntiles], F32), 'latcol': ([128, ntiles], F32),
        'a_s_rep': ([128, HID], F32), 'a_d_rep': ([128, HID], F32),
        'we_rep': ([128, H], F32), 'b_rep': ([128, HID], F32),
        'iota_row': ([128, 128], F32), 'ident': ([128, 128], F32),
        'ones_col': ([128, 1], F32),
    }
    if mlp:
        ins.update({'C1w': ([HID, FC], F32), 'C2w': ([128, 2 * FC], F32), 'C3w': ([128, 2], F32),
                    'c1b_col': ([128, 2], F32), 'c2b_col': ([128, 2], F32),
                    'pool_mat': ([128, BPC * NG], F32)})
    else:
        ins.update({'Wn': ([HID, HID], F32)})
    tin = _mk(ins, nc, "ExternalInput")
    if mlp:
        tout = _mk({'partials': ([NG, 1], F32)}, nc, "ExternalOutput")
    else:
        tout = _mk({'xp_next': ([BPC * BLK, HID], F32)}, nc, "ExternalOutput")

    with tile.TileContext(nc) as tc:
        with (
            tc.tile_pool(name="const", bufs=1) as constp,
            tc.tile_pool(name="gbuf", bufs=2) as gbufp,
            tc.tile_pool(name="work", bufs=3) as workp,
            tc.tile_pool(name="chunk", bufs=2) as chunkp,
            tc.tile_pool(name="slice", bufs=1) as slicep,
            tc.tile_pool(name="psA", bufs=2, space="PSUM") as psA,
            tc.tile_pool(name="psB", bufs=1, space="PSUM") as psB,
            tc.tile_pool(name="mlpp", bufs=2, space="PSUM") as mlpp,
        ):
            nc.gpsimd.load_library(library_config.mlp)
            c = {}
            for name in ['idx_lo', 'idx_hi', 'dstcol', 'latcol', 'a_s_rep', 'a_d_rep',
                         'we_rep', 'b_rep', 'iota_row', 'ident', 'ones_col'] + (
                         ['C1w', 'C2w', 'C3w', 'c1b_col', 'c2b_col', 'pool_mat'] if mlp else ['Wn']):
                shape, dt = ins[name]
                t = constp.tile(list(shape), dt, tag=name)
                nc.sync.dma_start(t[:], tin[name].ap())
                c[name] = t
            c['tab'] = tin['tab'].ap()

            # own xp slice -> SBUF; s_dst per block
            xpown = slicep.tile([128, BPC, HID], F32, tag="xpown")
            nc.sync.dma_start(xpown[:], tin['xp_own'].ap().rearrange("(b p) j -> p b j", p=128))
            sdst = slicep.tile([128, BPC, H], F32, tag="sdst")
            for b in range(BPC):
                t = workp.tile([128, HID], F32, tag="xa")
                nc.vector.tensor_tensor(out=t[:], in0=xpown[:, b, :], in1=c['a_d_rep'][:], op=ALU.mult)
                nc.vector.tensor_reduce(out=sdst[:, b, :], in_=t[:].rearrange("p (h c) -> p h c", h=H),
                                        op=ALU.add, axis=AX.X)

            xslice = slicep.tile([128, BPC, HID], F32, tag="xslice")
            _edge_phase(tc, c, host, not mlp, sdst, xslice,
                        (constp, gbufp, workp, chunkp, psA, psB))

            if not mlp:
                xpn = slicep.tile([128, BPC, HID], F32, tag="xpn")
                for b in range(BPC):
                    tp = psA.tile([128, 128], F32, tag="tpsum")
                    nc.tensor.transpose(tp[:], xslice[:, b, :], c['ident'][:])
                    xT = workp.tile([128, HID], F32, tag="xT")
                    nc.scalar.copy(out=xT[:], in_=tp[:])
                    xpp = psB.tile([128, HID], F32, tag="opsum")
                    nc.tensor.matmul(xpp[:], xT[:], c['Wn'][:], start=True, stop=True)
                    nc.scalar.copy(out=xpn[:, b, :], in_=xpp[:])
                nc.sync.dma_start(tout['xp_next'].ap().rearrange("(b p) j -> p b j", p=128), xpn[:])
            else:
                gp = psB.tile([NG, 1], F32, tag="dpsum")  # reuse tag budget
                for b in range(BPC):
                    tp = psA.tile([128, 128], F32, tag="tpsum")
                    nc.tensor.transpose(tp[:], xslice[:, b, :], c['ident'][:])
                    xT = workp.tile([128, HID], F32, tag="xT")
                    nc.scalar.copy(out=xT[:], in_=tp[:])
                    h1 = []
                    for jh in range(2):
                        hp = mlpp.tile([128, 128], F32, tag="mlpp")
                        nc.tensor.matmul(hp[:], c['C1w'][:, jh * 128:(jh + 1) * 128], xT[:],
                                         start=True, stop=True)
                        hs = workp.tile([128, 128], F32, tag=f"h1_{jh}")
                        nc.vector.tensor_scalar(out=hs[:], in0=hp[:],
                                                scalar1=c['c1b_col'][:, jh:jh + 1],
                                                scalar2=0.0, op0=ALU.add, op1=ALU.max)
                        h1.append(hs)
                    h2 = []
                    for jh in range(2):
                        hp = mlpp.tile([128, 128], F32, tag="mlpp")
                        for kc in range(2):
                            nc.tensor.matmul(hp[:], c['C2w'][:, kc * FC + jh * 128:kc * FC + (jh + 1) * 128],
                                             h1[kc][:], start=(kc == 0), stop=(kc == 1))
                        hs = workp.tile([128, 128], F32, tag=f"h2_{jh}")
                        nc.vector.tensor_scalar(out=hs[:], in0=hp[:],
                                                scalar1=c['c2b_col'][:, jh:jh + 1],
                                                scalar2=0.0, op0=ALU.add, op1=ALU.max)
                        h2.append(hs)
                    nvp = psB.tile([128, 1], F32, tag="small1")
                    for kc in range(2):
                        nc.tensor.matmul(nvp[:], h2[kc][:], c['C3w'][:, kc:kc + 1],
                                         start=(kc == 0), stop=(kc == 1))
                    nv = workp.tile([128, 1], F32, tag="nv")
                    nc.vector.tensor_scalar(out=nv[:], in0=nvp[:], scalar1=float(host['C3b'][0]),
                                            scalar2=0.0, op0=ALU.add, op1=ALU.max)
                    nc.tensor.matmul(gp[:], c['pool_mat'][:, b * NG:(b + 1) * NG], nv[:],
                                     start=(b == 0), stop=(b == BPC - 1))
                pt = workp.tile([NG, 1], F32, tag="pt")
                nc.scalar.copy(out=pt[:], in_=gp[:])
                nc.sync.dma_start(tout['partials'].ap(), pt[:])
    nc.compile()
    return nc


def _build_feat(host):
    """Launch 0: xp0 own slice from raw features."""
    import concourse.bacc as bacc
    import concourse.mybir as mybir
    import concourse.tile as tile
    from concourse import library_config
    F32 = mybir.dt.float32
    ALU = mybir.AluOpType
    AX = mybir.AxisListType
    ACTF = mybir.ActivationFunctionType
    nc = bacc.Bacc("TRN2", target_bir_lowering=False, debug=False, num_devices=NCORES)
    NB = NPAD // 128
    ins = {
        'req_w': ([128, NB], F32), 'us_own': ([128, BPC], F32),
        'mask_ge15': ([128, NB], F32), 'mask_lt15': ([128, NB], F32),
        'onehot4T': ([4, BPC * BLK], F32), 'T0': ([4, HID], F32),
        'w16_rep': ([128, HID], F32), 'w17_rep': ([128, HID], F32),
        'ones_col': ([128, 1], F32),
    }
    tin = _mk(ins, nc, "ExternalInput")
    tout = _mk({'xp_next': ([BPC * BLK, HID], F32)}, nc, "ExternalOutput")
    n = float(N - NL)
    with tile.TileContext(nc) as tc:
        with (
            tc.tile_pool(name="const", bufs=1) as constp,
            tc.tile_pool(name="work", bufs=3) as workp,
            tc.tile_pool(name="slice", bufs=1) as slicep,
            tc.tile_pool(name="ps", bufs=2, space="PSUM") as ps,
        ):
            nc.gpsimd.load_library(library_config.mlp)
            c = {}
            for name in ins:
                shape, dt = ins[name]
                t = constp.tile(list(shape), dt, tag=name)
                nc.sync.dma_start(t[:], tin[name].ap())
                c[name] = t
            d = workp.tile([128, NB], F32, tag="d")
            nc.vector.tensor_tensor(out=d[:], in0=c['req_w'][:], in1=c['mask_ge15'][:], op=ALU.mult)
            col = workp.tile([128, 1], F32, tag="col")
            nc.vector.tensor_reduce(out=col[:], in_=d[:], op=ALU.add, axis=AX.X)
            tot = ps.tile([1, 1], F32, tag="tot")
            nc.tensor.matmul(tot[:], col[:], c['ones_col'][:], start=True, stop=True)
            mean = workp.tile([1, 1], F32, tag="mean")
            nc.vector.tensor_scalar(out=mean[:], in0=tot[:], scalar1=1.0 / n, scalar2=None, op0=ALU.mult)
            mean_col = workp.tile([128, 1], F32, tag="mean_col")
            nc.gpsimd.partition_broadcast(mean_col[:], mean[:])
            nc.vector.tensor_scalar(out=d[:], in0=c['req_w'][:], scalar1=mean_col[:, 0:1], scalar2=None, op0=ALU.subtract)
            nc.vector.tensor_tensor(out=d[:], in0=d[:], in1=c['mask_ge15'][:], op=ALU.mult)
            d2 = workp.tile([128, NB], F32, tag="d2")
            nc.vector.tensor_tensor(out=d2[:], in0=d[:], in1=d[:], op=ALU.mult)
            nc.vector.tensor_reduce(out=col[:], in_=d2[:], op=ALU.add, axis=AX.X)
            tot2 = ps.tile([1, 1], F32, tag="tot2")
            nc.tensor.matmul(tot2[:], col[:], c['ones_col'][:], start=True, stop=True)
            var = workp.tile([1, 1], F32, tag="var")
            nc.vector.tensor_scalar(out=var[:], in0=tot2[:], scalar1=1.0 / (n - 1.0), scalar2=None, op0=ALU.mult)
            std = workp.tile([1, 1], F32, tag="std")
            nc.scalar.activation(out=std[:], in_=var[:], func=ACTF.Sqrt)
            nc.vector.tensor_scalar(out=std[:], in0=std[:], scalar1=1e-6, scalar2=None, op0=ALU.add)
            rinv = workp.tile([1, 1], F32, tag="rinv")
            nc.vector.reciprocal(out=rinv[:], in_=std[:])
            rinv_col = workp.tile([128, 1], F32, tag="rinv_col")
            nc.gpsimd.partition_broadcast(rinv_col[:], rinv[:])
            rf = workp.tile([128, NB], F32, tag="rf")
            nc.vector.tensor_scalar(out=rf[:], in0=d[:], scalar1=rinv_col[:, 0:1], scalar2=None, op0=ALU.mult)
            raw15 = workp.tile([128, NB], F32, tag="raw15")
            nc.vector.tensor_tensor(out=raw15[:], in0=c['req_w'][:], in1=c['mask_lt15'][:], op=ALU.mult)
            nc.vector.tensor_tensor(out=rf[:], in0=rf[:], in1=raw15[:], op=ALU.add)

            xpn = slicep.tile([128, BPC, HID], F32, tag="xpn")
            for b in range(BPC):
                mm = ps.tile([128, HID], F32, tag="mm")
                nc.tensor.matmul(mm[:], c['onehot4T'][:, b * 128:(b + 1) * 128], c['T0'][:],
                                 start=True, stop=True)
                x0 = workp.tile([128, HID], F32, tag="x0")
                nc.scalar.copy(out=x0[:], in_=mm[:])
                t1 = workp.tile([128, HID], F32, tag="t1")
                nc.vector.tensor_scalar(out=t1[:], in0=c['w16_rep'][:], scalar1=rf[:, b:b + 1], scalar2=None, op0=ALU.mult)
                nc.vector.tensor_tensor(out=x0[:], in0=x0[:], in1=t1[:], op=ALU.add)
                nc.vector.tensor_scalar(out=t1[:], in0=c['w17_rep'][:], scalar1=c['us_own'][:, b:b + 1], scalar2=None, op0=ALU.mult)
                nc.vector.tensor_tensor(out=xpn[:, b, :], in0=x0[:], in1=t1[:], op=ALU.add)
            nc.sync.dma_start(tout['xp_next'].ap().rearrange("(b p) j -> p b j", p=128), xpn[:])
    nc.compile()
    return nc


def _run(nc, in_maps, want_time=False):
    from concourse.bass_utils import run_bass_kernel_spmd
    t0 = time.monotonic()
    res = run_bass_kernel_spmd(nc, in_maps, core_ids=list(range(NCORES)))
    wall = (time.monotonic() - t0) * 1e9
    t = res.exec_time_ns if res.exec_time_ns else None
    return res.results, (t if t else wall)


def kernel(**inputs):
    key = 'k'
    if key not in _cache:
        host = _build_host({k: np.asarray(v) for k, v in inputs.items()})
        _cache[key] = (host, _build_feat(host), _build_gat(host, mlp=False), _build_gat(host, mlp=True))
    host, p_feat, p_gat, p_mlp = _cache[key]

    shared = dict(iota_row=host['iota_row'], ident=host['ident'], ones_col=host['ones_col'])
    times = []

    # launch 0: features -> xp0 slices
    in_maps = []
    for k in range(NCORES):
        perm = host['perms'][k]
        in_maps.append(dict(
            req_w=np.ascontiguousarray(host['req_w_full'][:, perm]),
            us_own=np.ascontiguousarray(host['us_w_full'][:, k * BPC:(k + 1) * BPC]),
            mask_ge15=np.ascontiguousarray(host['mask_ge15'][:, perm]),
            mask_lt15=np.ascontiguousarray(host['mask_lt15'][:, perm]),
            onehot4T=host['onehot4T'][k], T0=host['T0'],
            w16_rep=host['w16_rep'], w17_rep=host['w17_rep'],
            ones_col=host['ones_col']))
    res, t = _run(p_feat, in_maps)
    times.append(t)
    xp = np.concatenate([res[k]['xp_next'] for k in range(NCORES)], axis=0)

    for li in range(4):
        L = host['layers'][li]
        mlp = (li == 3)
        in_maps = []
        for k in range(NCORES):
            m = dict(tab=xp, xp_own=np.ascontiguousarray(xp[k * BPC * BLK:(k + 1) * BPC * BLK]),
                     idx_lo=host['idx_lo'][k], idx_hi=host['idx_hi'][k],
                     dstcol=host['dstcol'][k], latcol=host['latcol'][k],
                     a_s_rep=L['a_s'], a_d_rep=L['a_d'],
                     we_rep=np.tile(L['we'][None, :], (128, 1)).astype(np.float32),
                     b_rep=np.tile(L['b'][None, :], (128, 1)).astype(np.float32), **shared)
            if mlp:
                m.update(C1w=host['C1w'],
                         C2w=np.ascontiguousarray(np.concatenate(
                             [host['C2w'][0:128], host['C2w'][128:256]], axis=1)),
                         C3w=np.ascontiguousarray(host['C3w'].reshape(2, 128).T),
                         c1b_col=np.ascontiguousarray(host['C1b'].reshape(2, 128).T),
                         c2b_col=np.ascontiguousarray(host['C2b'].reshape(2, 128).T),
                         pool_mat=host['pool_mat'][k])
            else:
                m.update(Wn=L['Wn'])
            in_maps.append(m)
        res, t = _run(p_mlp if mlp else p_gat, in_maps)
        times.append(t)
        if not mlp:
            xp = np.concatenate([res[k]['xp_next'] for k in range(NCORES)], axis=0)

    partials = sum(res[k]['partials'] for k in range(NCORES))
    out = (partials[:, 0] / np.maximum(host['cnt'], 1.0)).astype(np.float32)[:, None]
    kernel._last_times = times
    return out



# revision 2
# speedup vs baseline: 14307.8598x; 14307.8598x over previous
"""CriticSwapGNN Trainium2 kernel: 4-layer GAT + MLP head + graph mean pool.

Sharding: nodes in 128-blocks, 8 cores x 49 blocks (dst-range ownership).
Edges sorted by dst, per dst-block, split lo/hi by src half (int16 gather
indices), tiled 128/tile. Per layer-launch: edge phase (dma_gather of xp rows,
on-chip segment softmax via one-hot matmuls) + node phase (xp_next = x_next@W).
Host concatenates per-core xp slices between launches.
"""
import os
import sys
import time
import numpy as np

if '/opt/trn_rl_repo' not in sys.path:
    sys.path.insert(0, '/opt/trn_rl_repo')

N = 50000; E = 800000; F = 16; HID = 128; H = 4; C = 32; FC = 256; NL = 15; NG = 8
NCORES = 8
BLK = 128
BPC = 49                      # blocks per core (uniform; core 7 pads)
NPAD = NCORES * BPC * BLK     # 50176
HALF = 4 * BPC * BLK          # 25088 (cores 0-3 own lo half)
CHUNK_BLKS = 2

_cache = {}


def _build_host(inputs):
    import concourse.mybir as mybir  # noqa: F401  (path check)
    src = np.asarray(inputs['edge_index'][0], np.int64)
    dst = np.asarray(inputs['edge_index'][1], np.int64)
    lat = np.asarray(inputs['latency'], np.float32)

    # ---- per (core, block) edge lists, sorted by dst ----
    order = np.argsort(dst, kind='stable')
    es, ed, el = src[order], dst[order], lat[order]
    blk_of = ed // BLK
    blk_starts = np.searchsorted(blk_of, np.arange(NCORES * BPC + 1))
    per = {}
    tlo = np.zeros((NCORES, BPC), np.int64)
    thi = np.zeros((NCORES, BPC), np.int64)
    for k in range(NCORES):
        for b in range(BPC):
            g = k * BPC + b
            s_, e_ = blk_starts[g], blk_starts[g + 1]
            bs, bd, bl = es[s_:e_], ed[s_:e_] - g * BLK, el[s_:e_]
            lo = bs < HALF
            per[(k, b)] = (bs[lo], bd[lo], bl[lo], bs[~lo] - HALF, bd[~lo], bl[~lo])
            tlo[k, b] = -(-len(bs[lo]) // 128)
            thi[k, b] = -(-len(bs[~lo]) // 128)
    TLO = tlo.max(axis=0)     # uniform tile layout across cores
    THI = thi.max(axis=0)

    # chunk layout: blocks grouped CHUNK_BLKS at a time
    chunks = []
    b = 0
    while b < BPC:
        blks = list(range(b, min(b + CHUNK_BLKS, BPC)))
        chunks.append(blks)
        b += CHUNK_BLKS
    ntiles = int((TLO + THI).sum())

    # ---- pack per-core arrays in the uniform layout ----
    idx_lo = np.zeros((NCORES, 128, int(TLO.sum()) * 8), np.int16)
    idx_hi = np.zeros((NCORES, 128, int(THI.sum()) * 8), np.int16)
    dstcol = np.full((NCORES, 128, ntiles), BLK, np.float32)   # pad col -> 128
    latcol = np.zeros((NCORES, 128, ntiles), np.float32)
    lo_off = np.concatenate([[0], np.cumsum(TLO)])
    hi_off = np.concatenate([[0], np.cumsum(THI)])

    def wrap16(a):
        return np.tile(a.astype(np.int16).reshape(-1, 16).T, (8, 1))

    # tile order within the global tile axis: block-major, lo tiles then hi
    tile_pos = []
    for b in range(BPC):
        for t in range(int(TLO[b])):
            tile_pos.append(('lo', b, t))
        for t in range(int(THI[b])):
            tile_pos.append(('hi', b, t))
    tp_index = {v: i for i, v in enumerate(tile_pos)}

    for k in range(NCORES):
        for b in range(BPC):
            slo, dlo, llo, shi, dhi, lhi = per[(k, b)]
            for half, s_, d_, l_, T_, off in (
                    ('lo', slo, dlo, llo, TLO, lo_off), ('hi', shi, dhi, lhi, THI, hi_off)):
                nt = int(T_[b])
                cap = nt * 128
                sp = np.zeros(cap, np.int64)
                dp = np.full(cap, BLK, np.int64)
                lp = np.zeros(cap, np.float32)
                sp[:len(s_)] = s_
                dp[:len(d_)] = d_
                lp[:len(l_)] = l_
                if nt:
                    w = wrap16(sp)
                    if half == 'lo':
                        idx_lo[k][:, int(off[b]) * 8:(int(off[b]) + nt) * 8] = w
                    else:
                        idx_hi[k][:, int(off[b]) * 8:(int(off[b]) + nt) * 8] = w
                    for t in range(nt):
                        gi = tp_index[(half, b, t)]
                        dstcol[k][:, gi] = dp[t * 128:(t + 1) * 128]
                        latcol[k][:, gi] = lp[t * 128:(t + 1) * 128]

    # ---- features / weights folding (host: index prep + weight folding only) ----
    type_ids = np.asarray(inputs['type_ids'], np.int64)
    onehot4T = np.zeros((NCORES, 4, BPC * BLK), np.float32)
    for k in range(NCORES):
        sl = slice(k * BPC * BLK, (k + 1) * BPC * BLK)
        ids = np.full(BPC * BLK, -1, np.int64)
        n_real = max(0, min(N - k * BPC * BLK, BPC * BLK))
        ids[:n_real] = type_ids[k * BPC * BLK:k * BPC * BLK + n_real]
        for t in range(4):
            onehot4T[k, t] = (ids == t).astype(np.float32)

    def wrapnode(x):  # [N] -> [128, 392] node-major blocks, zero pad
        o = np.zeros(NPAD, np.float32)
        o[:N] = x
        return o.reshape(-1, 128).T.copy()   # node n=128b+p -> [p, b]

    req_w_full = wrapnode(np.asarray(inputs['requests'], np.float32))
    us_w_full = wrapnode(np.asarray(inputs['update_step'], np.float32))
    idx_node = np.arange(NPAD).reshape(-1, 128).T
    mask_ge15 = ((idx_node >= NL) & (idx_node < N)).astype(np.float32)
    mask_lt15 = (idx_node < NL).astype(np.float32)

    # per-core column perm: own blocks first
    perms = []
    for k in range(NCORES):
        own = np.arange(k * BPC, (k + 1) * BPC)
        rest = np.array([c for c in range(NPAD // 128) if c not in set(own)])
        perms.append(np.concatenate([own, rest]))

    def a_rep(a):  # [H,C] -> [128, HID] replicated rows
        return np.tile(a.reshape(1, HID).astype(np.float32), (128, 1))

    def we_fold(We, a_e):
        We = np.asarray(We, np.float32); a_e = np.asarray(a_e, np.float32)
        return np.array([(We[0, h * C:(h + 1) * C] * a_e[h]).sum() for h in range(H)], np.float32)

    W0 = np.asarray(inputs['W0'], np.float32)
    T0 = (np.asarray(inputs['emb'], np.float32) @ W0[:F]).astype(np.float32)
    layers = []
    layers.append(dict(a_s=a_rep(np.asarray(inputs['as0'])), a_d=a_rep(np.asarray(inputs['ad0'])),
                       we=we_fold(inputs['We0'], inputs['ae0']), b=np.asarray(inputs['b0'], np.float32),
                       Wn=np.asarray(inputs['Wh'][0], np.float32), relu=True))
    layers.append(dict(a_s=a_rep(np.asarray(inputs['ash'][0])), a_d=a_rep(np.asarray(inputs['adh'][0])),
                       we=we_fold(np.asarray(inputs['Weh'][0]).reshape(1, -1), inputs['aeh'][0]),
                       b=np.asarray(inputs['bh'][0], np.float32),
                       Wn=np.asarray(inputs['Wh'][1], np.float32), relu=True))
    layers.append(dict(a_s=a_rep(np.asarray(inputs['ash'][1])), a_d=a_rep(np.asarray(inputs['adh'][1])),
                       we=we_fold(np.asarray(inputs['Weh'][1]).reshape(1, -1), inputs['aeh'][1]),
                       b=np.asarray(inputs['bh'][1], np.float32),
                       Wn=np.asarray(inputs['Wf'], np.float32), relu=True))
    layers.append(dict(a_s=a_rep(np.asarray(inputs['asf'])), a_d=a_rep(np.asarray(inputs['adf'])),
                       we=we_fold(inputs['Wef'], inputs['aef']), b=np.asarray(inputs['bf'], np.float32),
                       Wn=np.eye(HID, dtype=np.float32), relu=False))

    batch = np.asarray(inputs['batch'], np.int64)
    pool_mat = np.zeros((NCORES, 128, BPC * NG), np.float32)
    cnt = np.zeros(NG, np.float64)
    np.add.at(cnt, batch, 1.0)
    for k in range(NCORES):
        for b in range(BPC):
            base = (k * BPC + b) * BLK
            for p in range(128):
                n_ = base + p
                if n_ < N:
                    pool_mat[k, p, b * NG + batch[n_]] = 1.0

    host = dict(
        TLO=TLO, THI=THI, chunks=chunks, ntiles=ntiles, lo_off=lo_off, hi_off=hi_off,
        tile_pos=tile_pos, idx_lo=idx_lo, idx_hi=idx_hi, dstcol=dstcol, latcol=latcol,
        onehot4T=onehot4T, req_w_full=req_w_full, us_w_full=us_w_full,
        mask_ge15=mask_ge15, mask_lt15=mask_lt15, perms=perms, T0=T0,
        w16_rep=np.tile(W0[F][None, :], (128, 1)).astype(np.float32),
        w17_rep=np.tile(W0[F + 1][None, :], (128, 1)).astype(np.float32),
        layers=layers, cnt=cnt, pool_mat=pool_mat,
        C1w=np.asarray(inputs['C1w'], np.float32), C1b=np.asarray(inputs['C1b'], np.float32),
        C2w=np.asarray(inputs['C2w'], np.float32), C2b=np.asarray(inputs['C2b'], np.float32),
        C3w=np.asarray(inputs['C3w'], np.float32), C3b=np.asarray(inputs['C3b'], np.float32),
        iota_row=np.tile(np.arange(128, dtype=np.float32)[None, :], (128, 1)),
        ident=np.eye(128, dtype=np.float32),
        ones_col=np.ones((128, 1), np.float32),
    )
    return host


# ---------------------------------------------------------------- programs
def _mk(name_shapes, nc, kind):
    out = {}
    import concourse.mybir as mybir
    for name, (shape, dt) in name_shapes.items():
        out[name] = nc.dram_tensor(name, list(shape), dt, kind=kind)
    return out


def _edge_phase(tc, c, host, relu, sdst, xslice, pools):
    """Edge phase: reads gather table (DRAM tensors c['tab']), writes xslice."""
    import concourse.mybir as mybir
    nc = tc.nc
    F32 = mybir.dt.float32
    ALU = mybir.AluOpType
    AX = mybir.AxisListType
    ACTF = mybir.ActivationFunctionType
    constp, gbufp, workp, chunkp, psA, psB = pools
    TLO, THI, lo_off, hi_off = host['TLO'], host['THI'], host['lo_off'], host['hi_off']
    tp_index = {v: i for i, v in enumerate(host['tile_pos'])}

    for blks in host['chunks']:
        glo = int(sum(TLO[b] for b in blks))
        ghi = int(sum(THI[b] for b in blks))
        Tch = glo + ghi
        g_lo = gbufp.tile([128, max(glo, 1), HID], F32, tag="g_lo")
        g_hi = gbufp.tile([128, max(ghi, 1), HID], F32, tag="g_hi")
        if glo:
            nc.gpsimd.dma_gather(g_lo[:, 0:glo, :], c['tab'][0:HALF, :],
                                 c['idx_lo'][:, int(lo_off[blks[0]]) * 8:(int(lo_off[blks[0]]) + glo) * 8],
                                 glo * 128, glo * 128, HID, single_packet=False)
        if ghi:
            nc.gpsimd.dma_gather(g_hi[:, 0:ghi, :], c['tab'][HALF:NPAD, :],
                                 c['idx_hi'][:, int(hi_off[blks[0]]) * 8:(int(hi_off[blks[0]]) + ghi) * 8],
                                 ghi * 128, ghi * 128, HID, single_packet=False)

        s_src = chunkp.tile([128, max(Tch, 1), H], F32, tag="s_src")
        s_dst_e = chunkp.tile([128, max(Tch, 1), H], F32, tag="s_dst_e")
        oh_ch = chunkp.tile([128, max(Tch, 1), 128], F32, tag="oh_ch")
        araw = chunkp.tile([128, max(Tch, 1), H], F32, tag="araw")
        wexp = chunkp.tile([128, max(Tch, 1), H], F32, tag="wexp")

        # chunk-local tile enumeration: (kind, block, gather slot, chunk slot)
        tl = []
        li = hi = 0
        for b in blks:
            for t in range(int(TLO[b])):
                tl.append(('lo', b, li, len(tl))); li += 1
            for t in range(int(THI[b])):
                tl.append(('hi', b, hi, len(tl))); hi += 1

        # pass A
        for half, b, g, t in tl:
            xg = (g_lo if half == 'lo' else g_hi)[:, g, :]
            gidx = int((lo_off[b] if half == 'lo' else hi_off[b]) + g - (lo_off[blks[0]] if half == 'lo' else hi_off[blks[0]])) if False else None
            # global tile index for dstcol/latcol
            ti = tp_index[(half, b, g - int((lo_off[b] - lo_off[blks[0]]) if half == 'lo' else (hi_off[b] - hi_off[blks[0]])))]
            xa = workp.tile([128, HID], F32, tag="xa")
            nc.vector.tensor_tensor(out=xa[:], in0=xg, in1=c['a_s_rep'][:], op=ALU.mult)
            nc.vector.tensor_reduce(out=s_src[:, t, :], in_=xa[:].rearrange("p (h c) -> p h c", h=H),
                                    op=ALU.add, axis=AX.X)
            oh = oh_ch[:, t, :]
            nc.vector.tensor_scalar(out=oh, in0=c['iota_row'][:], scalar1=c['dstcol'][:, ti:ti + 1],
                                    scalar2=None, op0=ALU.is_equal)
            tp = psA.tile([128, 128], F32, tag="tpsum")
            nc.tensor.transpose(tp[:], oh, c['ident'][:])
            ohT = workp.tile([128, 128], F32, tag="ohT")
            nc.scalar.copy(out=ohT[:], in_=tp[:])
            sp = psB.tile([128, H], F32, tag="spsum")
            nc.tensor.matmul(sp[:], ohT[:], sdst[:, b, :], start=True, stop=True)
            nc.scalar.copy(out=s_dst_e[:, t, :], in_=sp[:])

        # chunk araw pipeline
        lwslice = []
        for half, b, g, t in tl:
            ti = tp_index[(half, b, g - int((lo_off[b] - lo_off[blks[0]]) if half == 'lo' else (hi_off[b] - hi_off[blks[0]])))]
            lwslice.append(ti)
        latw = workp.tile([128, max(Tch, 1), H], F32, tag="latw")
        for j, ti in enumerate(lwslice):
            nc.vector.tensor_scalar(out=latw[:, j, :], in0=c['we_rep'][:],
                                    scalar1=c['latcol'][:, ti:ti + 1], scalar2=None, op0=ALU.mult)
        nc.vector.tensor_tensor(out=araw[:], in0=s_src[:], in1=s_dst_e[:], op=ALU.add)
        nc.vector.tensor_tensor(out=araw[:], in0=araw[:], in1=latw[:], op=ALU.add)
        lr = workp.tile([128, max(Tch, 1), H], F32, tag="lr")
        nc.vector.tensor_scalar(out=lr[:], in0=araw[:], scalar1=0.2, scalar2=None, op0=ALU.mult)
        nc.vector.tensor_tensor(out=araw[:], in0=araw[:], in1=lr[:], op=ALU.max)
        mx = workp.tile([128, H], F32, tag="mx")
        nc.vector.tensor_reduce(out=mx[:], in_=araw[:].rearrange("p t h -> p h t"), op=ALU.max, axis=AX.X)
        emx = workp.tile([128, H], F32, tag="emx")
        nc.scalar.activation(out=emx[:], in_=mx[:], func=ACTF.Exp)
        msum = psB.tile([1, H], F32, tag="small1")
        nc.tensor.matmul(msum[:], c['ones_col'][:], emx[:], start=True, stop=True)
        M_row = workp.tile([1, H], F32, tag="M_row")
        nc.scalar.activation(out=M_row[:], in_=msum[:], func=ACTF.Ln)
        M_rep = workp.tile([128, H], F32, tag="M_rep")
        nc.gpsimd.partition_broadcast(M_rep[:], M_row[:])
        nc.vector.tensor_tensor(out=araw[:], in0=araw[:],
                                in1=M_rep[:].rearrange("p h -> p () h").broadcast_to([128, max(Tch, 1), H]),
                                op=ALU.subtract)
        nc.scalar.activation(out=wexp[:], in_=araw[:], func=ACTF.Exp)

        # pass B per block
        for b in blks:
            bt = [v for v in tl if v[1] == b]
            dps = psB.tile([128, H], F32, tag="dpsum")
            ops = psB.tile([128, HID], F32, tag="opsum")
            for j, (half, _b, g, t) in enumerate(bt):
                xg = (g_lo if half == 'lo' else g_hi)[:, g, :]
                oh = oh_ch[:, t, :]
                nc.tensor.matmul(dps[:], oh, wexp[:, t, :], start=(j == 0), stop=(j == len(bt) - 1))
                wmsg = workp.tile([128, HID], F32, tag="wmsg")
                for h in range(H):
                    nc.scalar.activation(out=wmsg[:, h * C:(h + 1) * C], in_=xg[:, h * C:(h + 1) * C],
                                         func=ACTF.Copy, scale=wexp[:, t, h:h + 1])
                nc.tensor.matmul(ops[:], oh, wmsg[:], start=(j == 0), stop=(j == len(bt) - 1))
            den = workp.tile([128, H], F32, tag="den")
            nc.vector.tensor_scalar(out=den[:], in0=dps[:], scalar1=1e-16, scalar2=None, op0=ALU.add)
            recip = workp.tile([128, H], F32, tag="recip")
            nc.vector.reciprocal(out=recip[:], in_=den[:])
            xn = workp.tile([128, HID], F32, tag="xn")
            nc.vector.tensor_tensor(out=xn[:], in0=ops[:],
                                    in1=recip[:].rearrange("p h -> p h ()").broadcast_to([128, H, C]),
                                    op=ALU.mult)
            nc.vector.tensor_tensor(out=xn[:], in0=xn[:], in1=c['b_rep'][:], op=ALU.add)
            if relu:
                nc.scalar.activation(out=xslice[:, b, :], in_=xn[:], func=ACTF.Relu)
            else:
                nc.scalar.copy(out=xslice[:, b, :], in_=xn[:])


def _build_gat(host, mlp):
    """One GAT layer launch. mlp=False: node phase -> xp_next slice out.
    mlp=True: final layer + MLP + pool -> partials out."""
    import concourse.bacc as bacc
    import concourse.mybir as mybir
    import concourse.tile as tile
    from concourse import library_config
    F32 = mybir.dt.float32
    I16 = mybir.dt.int16
    ALU = mybir.AluOpType
    AX = mybir.AxisListType
    ACTF = mybir.ActivationFunctionType
    nc = bacc.Bacc("TRN2", target_bir_lowering=False, debug=False, num_devices=NCORES)

    nlo8, nhi8 = host['idx_lo'].shape[2], host['idx_hi'].shape[2]
    ntiles = host['ntiles']
    ins = {
        'tab': ([NPAD, HID], F32), 'xp_own': ([BPC * BLK, HID], F32),
        'idx_lo': ([128, nlo8], I16), 'idx_hi': ([128, nhi8], I16),
        'dstcol': ([128, ntiles], F32), 'latcol': ([128, ntiles], F32),
        'a_s_rep': ([128, HID], F32), 'a_d_rep': ([128, HID], F32),
        'we_rep': ([128, H], F32), 'b_rep': ([128, HID], F32),
        'iota_row': ([128, 128], F32), 'ident': ([128, 128], F32),
        'ones_col': ([128, 1], F32),
    }
    if mlp:
        ins.update({'C1w': ([HID, FC], F32), 'C2w': ([128, 2 * FC], F32), 'C3w': ([128, 2], F32),
                    'c1b_col': ([128, 2], F32), 'c2b_col': ([128, 2], F32),
                    'pool_mat': ([128, BPC * NG], F32)})
    else:
        ins.update({'Wn': ([HID, HID], F32)})
    tin = _mk(ins, nc, "ExternalInput")
    if mlp:
        tout = _mk({'partials': ([NG, 1], F32)}, nc, "ExternalOutput")
    else:
        tout = _mk({'xp_next': ([BPC * BLK, HID], F32)}, nc, "ExternalOutput")

    with tile.TileContext(nc) as tc:
        with (
            tc.tile_pool(name="const", bufs=1) as constp,
            tc.tile_pool(name="gbuf", bufs=2) as gbufp,
            tc.tile_pool(name="work", bufs=3) as workp,
            tc.tile_pool(name="chunk", bufs=2) as chunkp,
            tc.tile_pool(name="slice", bufs=1) as slicep,
            tc.tile_pool(name="psA", bufs=2, space="PSUM") as psA,
            tc.tile_pool(name="psB", bufs=1, space="PSUM") as psB,
            tc.tile_pool(name="mlpp", bufs=2, space="PSUM") as mlpp,
        ):
            nc.gpsimd.load_library(library_config.mlp)
            c = {}
            for name in ['idx_lo', 'idx_hi', 'dstcol', 'latcol', 'a_s_rep', 'a_d_rep',
                         'we_rep', 'b_rep', 'iota_row', 'ident', 'ones_col'] + (
                         ['C1w', 'C2w', 'C3w', 'c1b_col', 'c2b_col', 'pool_mat'] if mlp else ['Wn']):
                shape, dt = ins[name]
                t = constp.tile(list(shape), dt, tag=name)
                nc.sync.dma_start(t[:], tin[name].ap())
                c[name] = t
            c['tab'] = tin['tab'].ap()

            # own xp slice -> SBUF; s_dst per block
            xpown = slicep.tile([128, BPC, HID], F32, tag="xpown")
            nc.sync.dma_start(xpown[:], tin['xp_own'].ap().rearrange("(b p) j -> p b j", p=128))
            sdst = slicep.tile([128, BPC, H], F32, tag="sdst")
            for b in range(BPC):
                t = workp.tile([128, HID], F32, tag="xa")
                nc.vector.tensor_tensor(out=t[:], in0=xpown[:, b, :], in1=c['a_d_rep'][:], op=ALU.mult)
                nc.vector.tensor_reduce(out=sdst[:, b, :], in_=t[:].rearrange("p (h c) -> p h c", h=H),
                                        op=ALU.add, axis=AX.X)

            xslice = slicep.tile([128, BPC, HID], F32, tag="xslice")
            _edge_phase(tc, c, host, not mlp, sdst, xslice,
                        (constp, gbufp, workp, chunkp, psA, psB))

            if not mlp:
                xpn = slicep.tile([128, BPC, HID], F32, tag="xpn")
                for b in range(BPC):
                    tp = psA.tile([128, 128], F32, tag="tpsum")
                    nc.tensor.transpose(tp[:], xslice[:, b, :], c['ident'][:])
                    xT = workp.tile([128, HID], F32, tag="xT")
                    nc.scalar.copy(out=xT[:], in_=tp[:])
                    xpp = psB.tile([128, HID], F32, tag="opsum")
                    nc.tensor.matmul(xpp[:], xT[:], c['Wn'][:], start=True, stop=True)
                    nc.scalar.copy(out=xpn[:, b, :], in_=xpp[:])
                nc.sync.dma_start(tout['xp_next'].ap().rearrange("(b p) j -> p b j", p=128), xpn[:])
            else:
                gp = psB.tile([NG, 1], F32, tag="dpsum")  # reuse tag budget
                for b in range(BPC):
                    tp = psA.tile([128, 128], F32, tag="tpsum")
                    nc.tensor.transpose(tp[:], xslice[:, b, :], c['ident'][:])
                    xT = workp.tile([128, HID], F32, tag="xT")
                    nc.scalar.copy(out=xT[:], in_=tp[:])
                    h1 = []
                    for jh in range(2):
                        hp = mlpp.tile([128, 128], F32, tag="mlpp")
                        nc.tensor.matmul(hp[:], c['C1w'][:, jh * 128:(jh + 1) * 128], xT[:],
                                         start=True, stop=True)
                        hs = workp.tile([128, 128], F32, tag=f"h1_{jh}")
                        nc.vector.tensor_scalar(out=hs[:], in0=hp[:],
                                                scalar1=c['c1b_col'][:, jh:jh + 1],
                                                scalar2=0.0, op0=ALU.add, op1=ALU.max)
                        h1.append(hs)
                    h2 = []
                    for jh in range(2):
                        hp = mlpp.tile([128, 128], F32, tag="mlpp")
                        for kc in range(2):
                            nc.tensor.matmul(hp[:], c['C2w'][:, kc * FC + jh * 128:kc * FC + (jh + 1) * 128],
                                             h1[kc][:], start=(kc == 0), stop=(kc == 1))
                        hs = workp.tile([128, 128], F32, tag=f"h2_{jh}")
                        nc.vector.tensor_scalar(out=hs[:], in0=hp[:],
                                                scalar1=c['c2b_col'][:, jh:jh + 1],
                                                scalar2=0.0, op0=ALU.add, op1=ALU.max)
                        h2.append(hs)
                    nvp = psB.tile([128, 1], F32, tag="small1")
                    for kc in range(2):
                        nc.tensor.matmul(nvp[:], h2[kc][:], c['C3w'][:, kc:kc + 1],
                                         start=(kc == 0), stop=(kc == 1))
                    nv = workp.tile([128, 1], F32, tag="nv")
                    nc.vector.tensor_scalar(out=nv[:], in0=nvp[:], scalar1=float(host['C3b'][0]),
                                            scalar2=0.0, op0=ALU.add, op1=ALU.max)
                    nc.tensor.matmul(gp[:], c['pool_mat'][:, b * NG:(b + 1) * NG], nv[:],
                                     start=(b == 0), stop=(b == BPC - 1))
                pt = workp.tile([NG, 1], F32, tag="pt")
                nc.scalar.copy(out=pt[:], in_=gp[:])
                nc.sync.dma_start(tout['partials'].ap(), pt[:])
    nc.compile()
    return nc


def _build_feat(host):
    """Launch 0: xp0 own slice from raw features."""
    import concourse.bacc as bacc
    import concourse.mybir as mybir
    import concourse.tile as tile
    from concourse import library_config
    F32 = mybir.dt.float32
    ALU = mybir.AluOpType
    AX = mybir.AxisListType
    ACTF = mybir.ActivationFunctionType
    nc = bacc.Bacc("TRN2", target_bir_lowering=False, debug=False, num_devices=NCORES)
    NB = NPAD // 128
    ins = {
        'req_w': ([128, NB], F32), 'us_own': ([128, BPC], F32),
        'mask_ge15': ([128, NB], F32), 'mask_lt15': ([128, NB], F32),
        'onehot4T': ([4, BPC * BLK], F32), 'T0': ([4, HID], F32),
        'w16_rep': ([128, HID], F32), 'w17_rep': ([128, HID], F32),
        'ones_col': ([128, 1], F32),
    }
    tin = _mk(ins, nc, "ExternalInput")
    tout = _mk({'xp_next': ([BPC * BLK, HID], F32)}, nc, "ExternalOutput")
    n = float(N - NL)
    with tile.TileContext(nc) as tc:
        with (
            tc.tile_pool(name="const", bufs=1) as constp,
            tc.tile_pool(name="work", bufs=3) as workp,
            tc.tile_pool(name="slice", bufs=1) as slicep,
            tc.tile_pool(name="ps", bufs=2, space="PSUM") as ps,
        ):
            nc.gpsimd.load_library(library_config.mlp)
            c = {}
            for name in ins:
                shape, dt = ins[name]
                t = constp.tile(list(shape), dt, tag=name)
                nc.sync.dma_start(t[:], tin[name].ap())
                c[name] = t
            d = workp.tile([128, NB], F32, tag="d")
            nc.vector.tensor_tensor(out=d[:], in0=c['req_w'][:], in1=c['mask_ge15'][:], op=ALU.mult)
            col = workp.tile([128, 1], F32, tag="col")
            nc.vector.tensor_reduce(out=col[:], in_=d[:], op=ALU.add, axis=AX.X)
            tot = ps.tile([1, 1], F32, tag="tot")
            nc.tensor.matmul(tot[:], col[:], c['ones_col'][:], start=True, stop=True)
            mean = workp.tile([1, 1], F32, tag="mean")
            nc.vector.tensor_scalar(out=mean[:], in0=tot[:], scalar1=1.0 / n, scalar2=None, op0=ALU.mult)
            mean_col = workp.tile([128, 1], F32, tag="mean_col")
            nc.gpsimd.partition_broadcast(mean_col[:], mean[:])
            nc.vector.tensor_scalar(out=d[:], in0=c['req_w'][:], scalar1=mean_col[:, 0:1], scalar2=None, op0=ALU.subtract)
            nc.vector.tensor_tensor(out=d[:], in0=d[:], in1=c['mask_ge15'][:], op=ALU.mult)
            d2 = workp.tile([128, NB], F32, tag="d2")
            nc.vector.tensor_tensor(out=d2[:], in0=d[:], in1=d[:], op=ALU.mult)
            nc.vector.tensor_reduce(out=col[:], in_=d2[:], op=ALU.add, axis=AX.X)
            tot2 = ps.tile([1, 1], F32, tag="tot2")
            nc.tensor.matmul(tot2[:], col[:], c['ones_col'][:], start=True, stop=True)
            var = workp.tile([1, 1], F32, tag="var")
            nc.vector.tensor_scalar(out=var[:], in0=tot2[:], scalar1=1.0 / (n - 1.0), scalar2=None, op0=ALU.mult)
            std = workp.tile([1, 1], F32, tag="std")
            nc.scalar.activation(out=std[:], in_=var[:], func=ACTF.Sqrt)
            nc.vector.tensor_scalar(out=std[:], in0=std[:], scalar1=1e-6, scalar2=None, op0=ALU.add)
            rinv = workp.tile([1, 1], F32, tag="rinv")
            nc.vector.reciprocal(out=rinv[:], in_=std[:])
            rinv_col = workp.tile([128, 1], F32, tag="rinv_col")
            nc.gpsimd.partition_broadcast(rinv_col[:], rinv[:])
            rf = workp.tile([128, NB], F32, tag="rf")
            nc.vector.tensor_scalar(out=rf[:], in0=d[:], scalar1=rinv_col[:, 0:1], scalar2=None, op0=ALU.mult)
            raw15 = workp.tile([128, NB], F32, tag="raw15")
            nc.vector.tensor_tensor(out=raw15[:], in0=c['req_w'][:], in1=c['mask_lt15'][:], op=ALU.mult)
            nc.vector.tensor_tensor(out=rf[:], in0=rf[:], in1=raw15[:], op=ALU.add)

            xpn = slicep.tile([128, BPC, HID], F32, tag="xpn")
            for b in range(BPC):
                mm = ps.tile([128, HID], F32, tag="mm")
                nc.tensor.matmul(mm[:], c['onehot4T'][:, b * 128:(b + 1) * 128], c['T0'][:],
                                 start=True, stop=True)
                x0 = workp.tile([128, HID], F32, tag="x0")
                nc.scalar.copy(out=x0[:], in_=mm[:])
                t1 = workp.tile([128, HID], F32, tag="t1")
                nc.vector.tensor_scalar(out=t1[:], in0=c['w16_rep'][:], scalar1=rf[:, b:b + 1], scalar2=None, op0=ALU.mult)
                nc.vector.tensor_tensor(out=x0[:], in0=x0[:], in1=t1[:], op=ALU.add)
                nc.vector.tensor_scalar(out=t1[:], in0=c['w17_rep'][:], scalar1=c['us_own'][:, b:b + 1], scalar2=None, op0=ALU.mult)
                nc.vector.tensor_tensor(out=xpn[:, b, :], in0=x0[:], in1=t1[:], op=ALU.add)
            nc.sync.dma_start(tout['xp_next'].ap().rearrange("(b p) j -> p b j", p=128), xpn[:])
    nc.compile()
    return nc


def _run(nc, in_maps, want_time=False):
    from concourse.bass_utils import run_bass_kernel_spmd
    t0 = time.monotonic()
    res = run_bass_kernel_spmd(nc, in_maps, core_ids=list(range(NCORES)))
    wall = (time.monotonic() - t0) * 1e9
    t = res.exec_time_ns if res.exec_time_ns else None
    _run.last_results.append(res)
    return res.results, (t if t else wall)


_run.last_results = []


def kernel(**inputs):
    key = 'k'
    if key not in _cache:
        host = _build_host({k: np.asarray(v) for k, v in inputs.items()})
        _cache[key] = (host, _build_feat(host), _build_gat(host, mlp=False), _build_gat(host, mlp=True))
    host, p_feat, p_gat, p_mlp = _cache[key]

    shared = dict(iota_row=host['iota_row'], ident=host['ident'], ones_col=host['ones_col'])
    times = []

    # launch 0: features -> xp0 slices
    in_maps = []
    for k in range(NCORES):
        perm = host['perms'][k]
        in_maps.append(dict(
            req_w=np.ascontiguousarray(host['req_w_full'][:, perm]),
            us_own=np.ascontiguousarray(host['us_w_full'][:, k * BPC:(k + 1) * BPC]),
            mask_ge15=np.ascontiguousarray(host['mask_ge15'][:, perm]),
            mask_lt15=np.ascontiguousarray(host['mask_lt15'][:, perm]),
            onehot4T=host['onehot4T'][k], T0=host['T0'],
            w16_rep=host['w16_rep'], w17_rep=host['w17_rep'],
            ones_col=host['ones_col']))
    res, t = _run(p_feat, in_maps)
    times.append(t)
    xp = np.concatenate([res[k]['xp_next'] for k in range(NCORES)], axis=0)

    for li in range(4):
        L = host['layers'][li]
        mlp = (li == 3)
        in_maps = []
        for k in range(NCORES):
            m = dict(tab=xp, xp_own=np.ascontiguousarray(xp[k * BPC * BLK:(k + 1) * BPC * BLK]),
                     idx_lo=host['idx_lo'][k], idx_hi=host['idx_hi'][k],
                     dstcol=host['dstcol'][k], latcol=host['latcol'][k],
                     a_s_rep=L['a_s'], a_d_rep=L['a_d'],
                     we_rep=np.tile(L['we'][None, :], (128, 1)).astype(np.float32),
                     b_rep=np.tile(L['b'][None, :], (128, 1)).astype(np.float32), **shared)
            if mlp:
                m.update(C1w=host['C1w'],
                         C2w=np.ascontiguousarray(np.concatenate(
                             [host['C2w'][0:128], host['C2w'][128:256]], axis=1)),
                         C3w=np.ascontiguousarray(host['C3w'].reshape(2, 128).T),
                         c1b_col=np.ascontiguousarray(host['C1b'].reshape(2, 128).T),
                         c2b_col=np.ascontiguousarray(host['C2b'].reshape(2, 128).T),
                         pool_mat=host['pool_mat'][k])
            else:
                m.update(Wn=L['Wn'])
            in_maps.append(m)
        res, t = _run(p_mlp if mlp else p_gat, in_maps)
        times.append(t)
        if not mlp:
            xp = np.concatenate([res[k]['xp_next'] for k in range(NCORES)], axis=0)

    partials = sum(res[k]['partials'] for k in range(NCORES))
    out = (partials[:, 0] / np.maximum(host['cnt'], 1.0)).astype(np.float32)[:, None]
    kernel._last_times = times
    return out



# revision 15
# speedup vs baseline: 36160.6442x; 2.5273x over previous
"""CriticSwapGNN Trainium2 kernel v4: 4-layer GAT + MLP head + graph mean pool.

Design:
- Per-head orthogonal rotation Q of each 32-dim head subspace so that
  a_s -> coef_s*e0 and a_d -> cd0*e0 + cd1*e1.  Attention scores become
  strided column reads of the (rotated) feature table; the un-rotation
  Q^T folds into the node-phase matmul (hidden layers) or into C1w (final).
- Dst-aligned edge layout: nodes permuted per core by (nlo, nhi) snake
  order; blocks of 128 dsts with uniform k-rectangles (lo/hi src halves
  for int16 gather indices).  Segment softmax is exact per-dst via slab
  ops; the scatter is K psum-accumulating matmuls out[f,d] += wm_k^T.
- bf16 feature table + gathers (256B rows), bf16 matmuls, fp32 psum.
- 5 launches: feat, 3x gat, gat+mlp+pool; host re-assembles the table
  (with zero pad rows per half for deg-0/pad dsts) between launches.
"""
import os
import sys
import time
import numpy as np
import ml_dtypes

if '/opt/trn_rl_repo' not in sys.path:
    sys.path.insert(0, '/opt/trn_rl_repo')

N = 50000; E = 800000; F = 16; HID = 128; H = 4; C = 32; FC = 256; NL = 15; NG = 8
NCORES = 8
BLK = 128
BPC = 49                      # blocks per core
NPC = BPC * BLK               # 6272 nodes per core
NPAD = NCORES * NPC           # 50176
HALF = NPAD // 2              # 25088
ZPAD = 8                      # zero rows appended per half
HROWS = HALF + ZPAD           # rows per half in the table
ZIDX = HALF                   # index of the zero row (relative to half)
MAXCOL = 128                  # max gather columns (lo+hi) per chunk
BF = ml_dtypes.bfloat16

_cache = {}


def _bf16(x):
    return np.asarray(x, np.float32).astype(BF)


def _make_q(a_s, a_d):
    """Per-head Q [C,C]; returns Qblk [HID,HID] f32 + score coefs."""
    Qb = np.zeros((HID, HID), np.float32)
    cs = np.zeros(H, np.float32)
    cd0 = np.zeros(H, np.float32)
    cd1 = np.zeros(H, np.float32)
    for h in range(H):
        a1 = np.asarray(a_s[h], np.float64)
        a2 = np.asarray(a_d[h], np.float64)
        q1 = a1 / np.linalg.norm(a1)
        a2o = a2 - (a2 @ q1) * q1
        n2 = np.linalg.norm(a2o)
        if n2 < 1e-12:
            v = np.zeros(C); v[np.argmin(np.abs(q1))] = 1.0
            a2o = v - (v @ q1) * q1
            n2 = np.linalg.norm(a2o)
        q2 = a2o / n2
        M = np.random.default_rng(h).standard_normal((C, C))
        M[:, 0] = q1; M[:, 1] = q2
        Q, _ = np.linalg.qr(M)
        Q[:, 0] = q1; Q[:, 1] = q2
        for j in range(2, C):
            v = Q[:, j]
            for i in range(j):
                v = v - (v @ Q[:, i]) * Q[:, i]
            Q[:, j] = v / np.linalg.norm(v)
        Qb[h * C:(h + 1) * C, h * C:(h + 1) * C] = Q.astype(np.float32)
        cs[h] = a1 @ q1
        cd0[h] = a2 @ q1
        cd1[h] = a2 @ q2
    return Qb, cs, cd0, cd1


def _wrap16(a):
    """Flat idx list -> [128, n/16] int16 (wrapped 16, replicated x8)."""
    return np.tile(np.asarray(a, np.int16).reshape(-1, 16).T, (8, 1))


def _build_host(inputs):
    src = np.asarray(inputs['edge_index'][0], np.int64)
    dst = np.asarray(inputs['edge_index'][1], np.int64)
    lat = np.asarray(inputs['latency'], np.float32)

    # ---- node permutation: per HALF, snake sort by (nlo, nhi), deal to
    # the half's 4 cores by adjacent 128-groups (aligned K quantiles) ----
    nlo_c = np.zeros(N, np.int64); np.add.at(nlo_c, dst[src < HALF], 1)
    nhi_c = np.zeros(N, np.int64); np.add.at(nhi_c, dst[src >= HALF], 1)
    perm2 = np.zeros(N, np.int64)
    for half in range(2):
        lo_n = half * HALF
        hi_n = min((half + 1) * HALF, N)
        olds = np.arange(lo_n, hi_n)
        nl, nh = nlo_c[olds], nhi_c[olds]
        order = np.lexsort((nh, nl))
        segs = []
        vals = nl[order]
        for v in np.unique(vals):
            seg = order[vals == v]
            if v % 2 == 1:
                seg = seg[::-1]
            segs.append(seg)
        order = np.concatenate(segs)
        o = olds[order]
        i = np.arange(len(o))
        kq = (i // BLK) % 4
        b_ = i // (BLK * 4)
        p_ = i % BLK
        perm2[o] = (4 * half + kq) * NPC + b_ * BLK + p_

    # edge lists in final id space
    src_n, dst_n = perm2[src], perm2[dst]
    order_e = np.lexsort((src_n, dst_n))
    es, ed, el = src_n[order_e], dst_n[order_e], lat[order_e]
    starts = np.searchsorted(ed, np.arange(NPAD + 1))

    klo_all = np.zeros((NCORES, BPC), np.int64)
    khi_all = np.zeros((NCORES, BPC), np.int64)
    pp = {}
    for k in range(NCORES):
        for b in range(BPC):
            base = (k * BPC + b) * BLK
            klo = khi = 0
            lst = []
            for p in range(BLK):
                nd = base + p
                s_, e_ = starts[nd], starts[nd + 1]
                ss, ll = es[s_:e_], el[s_:e_]
                m = ss < HALF
                lst.append((ss[m], ll[m], ss[~m] - HALF, ll[~m]))
                klo = max(klo, int(m.sum())); khi = max(khi, int((~m).sum()))
            klo_all[k, b] = klo; khi_all[k, b] = khi
            pp[(k, b)] = lst

    KLO = klo_all.max(axis=0).astype(int)
    KHI = khi_all.max(axis=0).astype(int)
    colK = KLO + KHI                         # ragged block columns
    rg0 = np.concatenate([[0], np.cumsum(colK)]).astype(int)
    ilo_off = np.concatenate([[0], np.cumsum(KLO)]).astype(int)
    ihi_off = np.concatenate([[0], np.cumsum(KHI)]).astype(int)
    ncol = int(rg0[-1])
    ncol_lo = int(ilo_off[-1]); ncol_hi = int(ihi_off[-1])

    # chunks: consecutive blocks, sum of rag cols <= MAXCOL (buffering only)
    chunks = []   # (b0, cn)
    b0 = 0
    while b0 < BPC:
        cn = 1
        while (b0 + cn < BPC and cn < 8 and
               rg0[b0 + cn + 1] - rg0[b0] <= MAXCOL):
            cn += 1
        chunks.append((b0, cn))
        b0 += cn

    # ---- pack gather idx + lat/pad (ragged layout) ----
    idx_lo = np.full((NCORES, ncol_lo * BLK), ZIDX, np.int64)
    idx_hi = np.full((NCORES, ncol_hi * BLK), ZIDX, np.int64)
    latpack = np.zeros((NCORES, 128, ncol), np.float32)
    padmask = np.ones((NCORES, 128, ncol), np.float32)
    for k in range(NCORES):
        for b in range(BPC):
            lst = pp[(k, b)]
            kl, kh = KLO[b], KHI[b]
            for p in range(BLK):
                sl, ll, sh, lh = lst[p]
                assert len(sl) <= kl and len(sh) <= kh
                gl0 = ilo_off[b]
                cl0 = rg0[b]
                for t in range(len(sl)):
                    idx_lo[k, (gl0 + t) * BLK + p] = sl[t]
                    latpack[k, p, cl0 + t] = ll[t]
                    padmask[k, p, cl0 + t] = 0.0
                gh0 = ihi_off[b]
                ch0 = rg0[b] + kl
                for t in range(len(sh)):
                    idx_hi[k, (gh0 + t) * BLK + p] = sh[t]
                    latpack[k, p, ch0 + t] = lh[t]
                    padmask[k, p, ch0 + t] = 0.0
    idx_lo16 = np.stack([_wrap16(idx_lo[k]) for k in range(NCORES)])
    idx_hi16 = np.stack([_wrap16(idx_hi[k]) for k in range(NCORES)])

    # ---- weights / folds ----
    def getf(x):
        return np.asarray(x, np.float32)

    def wefold(We, a_e):
        We = getf(We).reshape(-1); a_e = getf(a_e)
        return np.array([(We[h * C:(h + 1) * C] * a_e[h]).sum() for h in range(H)],
                        np.float32)

    Ws = [getf(inputs['W0']), getf(inputs['Wh'][0]), getf(inputs['Wh'][1]),
          getf(inputs['Wf'])]
    bs = [getf(inputs['b0']), getf(inputs['bh'][0]), getf(inputs['bh'][1]),
          getf(inputs['bf'])]
    a_ss = [getf(inputs['as0']), getf(inputs['ash'][0]), getf(inputs['ash'][1]),
            getf(inputs['asf'])]
    a_ds = [getf(inputs['ad0']), getf(inputs['adh'][0]), getf(inputs['adh'][1]),
            getf(inputs['adf'])]
    wes = [wefold(inputs['We0'], inputs['ae0']),
           wefold(inputs['Weh'][0], inputs['aeh'][0]),
           wefold(inputs['Weh'][1], inputs['aeh'][1]),
           wefold(inputs['Wef'], inputs['aef'])]
    QQ = [_make_q(a_ss[i], a_ds[i]) for i in range(4)]
    Qs = [q[0] for q in QQ]
    # per-layer column scaling D: col h*C -> cs[h], col h*C+1 -> s1[h];
    # s_src = col0 directly, s_dst = c0p*col0 + c1p*col1; un-rotation rows
    # are scaled by 1/D to compensate exactly.
    Dv = []; c0p = []; c1p = []
    for li in range(4):
        _, cs, cd0, cd1 = QQ[li]
        s1 = np.where(np.abs(cd1) > 1e-3, cd1, 1.0).astype(np.float32)
        dv = np.ones(HID, np.float32)
        dv[0::C] = cs
        dv[1::C] = s1
        Dv.append(dv)
        c0p.append((cd0 / cs).astype(np.float32))
        c1p.append((cd1 / s1).astype(np.float32))

    layers = []
    for li in range(4):
        s = latpack[:, :, :, None] * wes[li][None, None, None, :]
        s = s * (1.0 - padmask[:, :, :, None]) + padmask[:, :, :, None] * (-30000.0)
        d = dict(c0p_rep=np.tile(c0p[li][None, :], (128, 1)).astype(np.float32),
                 c1p_rep=np.tile(c1p[li][None, :], (128, 1)).astype(np.float32),
                 slat=_bf16(s))
        if li < 3:
            d['QT'] = _bf16((Qs[li].T) / Dv[li][:, None])
            d['Wq'] = _bf16((Ws[li + 1] @ Qs[li + 1]) * Dv[li + 1][None, :])
            d['b_col'] = bs[li].reshape(128, 1).astype(np.float32)
        layers.append(d)

    C1w = getf(inputs['C1w']); C1b = getf(inputs['C1b'])
    C2w = getf(inputs['C2w']); C2b = getf(inputs['C2b'])
    C3w = getf(inputs['C3w']); C3b = getf(inputs['C3b'])
    c1bp = (bs[3] @ C1w + C1b).astype(np.float32)
    C2p = np.zeros((128, 4 * 128), np.float32)
    for kc in range(2):
        for jh in range(2):
            C2p[:, (2 * kc + jh) * 128:(2 * kc + jh + 1) * 128] = \
                C2w[kc * 128:(kc + 1) * 128, jh * 128:(jh + 1) * 128]
    mlp = dict(QC1=_bf16((Qs[3].T / Dv[3][:, None]) @ C1w),
               c1b_col=np.ascontiguousarray(c1bp.reshape(2, 128).T),
               C2w=_bf16(C2p),
               c2b_col=np.ascontiguousarray(C2b.reshape(2, 128).T.astype(np.float32)),
               C3w=_bf16(np.ascontiguousarray(C3w.reshape(2, 128).T)),
               c3b=float(C3b[0]))

    # ---- feature phase arrays (permuted by perm2) ----
    type_ids = np.asarray(inputs['type_ids'], np.int64)
    us = np.asarray(inputs['update_step'], np.float32)
    req = np.asarray(inputs['requests'], np.float32)
    newpos = perm2

    def wrapnode(x_old):
        o = np.zeros(NPAD, np.float32)
        o[newpos[:N]] = x_old[:N]
        return o.reshape(-1, 128).T.copy()

    req_w_full = wrapnode(req)
    us_w_full = wrapnode(us)
    isreal = np.zeros(NPAD, np.float32); isreal[newpos[:N]] = 1.0
    islt15 = np.zeros(NPAD, np.float32); islt15[newpos[:NL]] = 1.0
    mask_ge15 = (isreal - islt15).reshape(-1, 128).T.copy()
    mask_lt15 = islt15.reshape(-1, 128).T.copy()
    onehot4T = np.zeros((NCORES, 4, NPC), np.float32)
    nn_all = newpos[:N]
    onehot4T[nn_all // NPC, type_ids[:N], nn_all % NPC] = 1.0

    W0q = (Ws[0] @ Qs[0]) * Dv[0][None, :]
    T0q = (getf(inputs['emb']) @ W0q[:F]).astype(np.float32)

    batch = np.asarray(inputs['batch'], np.int64)
    pool_mat = np.zeros((NCORES, 128, BPC * NG), np.float32)
    cnt = np.zeros(NG, np.float64)
    np.add.at(cnt, batch, 1.0)
    nn_new = newpos[np.arange(N)]
    kk_ = nn_new // NPC
    b_ = (nn_new % NPC) // BLK
    p_ = nn_new % BLK
    for old in range(N):
        pool_mat[kk_[old], p_[old], b_[old] * NG + batch[old]] = 1.0

    host = dict(
        chunks=chunks, ncol=ncol, ncol_lo=ncol_lo, ncol_hi=ncol_hi,
        KLO=KLO, KHI=KHI, colK=colK, rg0=rg0,
        ilo_off=ilo_off, ihi_off=ihi_off,
        idx_lo16=idx_lo16, idx_hi16=idx_hi16,
        nlo_idx=idx_lo.shape[1], nhi_idx=idx_hi.shape[1],
        layers=layers, mlp=mlp, cnt=cnt, pool_mat=pool_mat,
        onehot4T=onehot4T, T0q=T0q,
        w16q_rep=np.tile(W0q[F][None, :], (128, 1)).astype(np.float32),
        w17q_rep=np.tile(W0q[F + 1][None, :], (128, 1)).astype(np.float32),
        req_w_full=req_w_full, us_w_full=us_w_full,
        mask_ge15=mask_ge15, mask_lt15=mask_lt15,
        ident_bf=np.eye(128, dtype=np.float32).astype(BF),
        ones_col=np.ones((128, 1), np.float32),
    )
    return host


def _mk(name_shapes, nc, kind):
    out = {}
    for name, (shape, dt) in name_shapes.items():
        out[name] = nc.dram_tensor(name, list(shape), dt, kind=kind)
    return out


def _build_gat(host, mlp):
    import concourse.bacc as bacc
    import concourse.mybir as mybir
    import concourse.tile as tile
    from concourse import library_config
    F32 = mybir.dt.float32
    BF16 = mybir.dt.bfloat16
    I16 = mybir.dt.int16
    ALU = mybir.AluOpType
    AX = mybir.AxisListType
    ACTF = mybir.ActivationFunctionType
    nc = bacc.Bacc("TRN2", target_bir_lowering=False, debug=False,
                   num_devices=NCORES)

    chunks = host['chunks']
    ncol = host['ncol']
    KLO, KHI, colK, rg0 = host['KLO'], host['KHI'], host['colK'], host['rg0']
    ilo_off, ihi_off = host['ilo_off'], host['ihi_off']
    ins = {
        'tab': ([2 * HROWS, HID], BF16),
        'xp_own': ([NPC, HID], BF16),
        'idx_lo': ([128, host['nlo_idx'] // 16], I16),
        'idx_hi': ([128, host['nhi_idx'] // 16], I16),
        'slat': ([128, ncol, H], BF16),
        'c0p_rep': ([128, H], F32), 'c1p_rep': ([128, H], F32),
        'ident_bf': ([128, 128], BF16),
    }
    if mlp:
        ins.update({'QC1': ([HID, FC], BF16), 'C2w': ([128, 4 * 128], BF16),
                    'C3w': ([128, 2], BF16),
                    'c1b_col': ([128, 2], F32), 'c2b_col': ([128, 2], F32),
                    'pool_mat': ([128, BPC * NG], F32)})
    else:
        ins.update({'QT': ([HID, HID], BF16), 'Wq': ([HID, HID], BF16),
                    'b_col': ([128, 1], F32)})
    tin = _mk(ins, nc, "ExternalInput")
    if mlp:
        tout = _mk({'partials': ([NG, 1], F32)}, nc, "ExternalOutput")
    else:
        tout = _mk({'xp_next': ([NPC, HID], BF16)}, nc, "ExternalOutput")

    c3b = host['mlp']['c3b'] if mlp else 0.0

    with tile.TileContext(nc) as tc:
        with (
            tc.tile_pool(name="const", bufs=1) as constp,
            tc.tile_pool(name="gbuf", bufs=2) as gbufp,
            tc.tile_pool(name="wmbuf", bufs=2) as wmbufp,
            tc.tile_pool(name="slab", bufs=2) as slabp,
            tc.tile_pool(name="work", bufs=3) as workp,
            tc.tile_pool(name="slice", bufs=1) as slicep,
            tc.tile_pool(name="psS", bufs=4, space="PSUM") as psS,
            tc.tile_pool(name="psN", bufs=(1 if mlp else 2), space="PSUM") as psN,
        ):
            nc.gpsimd.load_library(library_config.mlp)
            cst = {}
            for name in ins:
                if name in ('tab', 'xp_own'):
                    continue
                shape, dt = ins[name]
                t = constp.tile(list(shape), dt, tag=name)
                nc.sync.dma_start(t[:], tin[name].ap())
                cst[name] = t
            tab_lo = tin['tab'].ap()[0:HROWS, :]
            tab_hi = tin['tab'].ap()[HROWS:2 * HROWS, :]

            # own slice -> sdst_all [128, BPC, H]
            xpo = slicep.tile([128, BPC, HID], BF16, tag="xpo")
            nc.sync.dma_start(xpo[:],
                              tin['xp_own'].ap().rearrange("(b p) j -> p b j", p=128))
            sdst_all = slicep.tile([128, BPC, H], F32, tag="sdst")
            t0 = workp.tile([128, BPC, H], F32, tag="t0")
            nc.vector.tensor_tensor(
                out=t0[:], in0=xpo[:, :, 0:HID:C],
                in1=cst['c0p_rep'][:].rearrange("p h -> p () h").broadcast_to([128, BPC, H]),
                op=ALU.mult)
            nc.vector.tensor_tensor(
                out=sdst_all[:], in0=xpo[:, :, 1:HID:C],
                in1=cst['c1p_rep'][:].rearrange("p h -> p () h").broadcast_to([128, BPC, H]),
                op=ALU.mult)
            nc.vector.tensor_tensor(out=sdst_all[:], in0=sdst_all[:], in1=t0[:],
                                    op=ALU.add)

            gp = psN.tile([NG, 1], F32, tag="pool", name="gp") if mlp else None
            xpn = None if mlp else slicep.tile([128, BPC, HID], BF16, tag="xpn",
                                               name="xpn")

            for ci, (b0, cn) in enumerate(chunks):
                nct = int(rg0[b0 + cn] - rg0[b0])
                g = gbufp.tile([128, MAXCOL, HID], BF16, tag="g")
                for bi in range(cn):
                    b = b0 + bi
                    r0 = int(rg0[b] - rg0[b0])
                    kl, kh = int(KLO[b]), int(KHI[b])
                    if kl:
                        o0 = int(ilo_off[b])
                        nc.gpsimd.dma_gather(
                            g[:, r0:r0 + kl, :], tab_lo,
                            cst['idx_lo'][:, o0 * 8:(o0 + kl) * 8],
                            kl * 128, kl * 128, HID, single_packet=False)
                    if kh:
                        o0 = int(ihi_off[b])
                        nc.gpsimd.dma_gather(
                            g[:, r0 + kl:r0 + kl + kh, :], tab_hi,
                            cst['idx_hi'][:, o0 * 8:(o0 + kh) * 8],
                            kh * 128, kh * 128, HID, single_packet=False)

                cb = int(rg0[b0])
                araw = slabp.tile([128, MAXCOL, H], F32, tag="araw")
                # per-block: araw = g[:, :, 0::C] + sdst_b (bcast over k)
                for bi in range(cn):
                    b = b0 + bi
                    r0 = int(rg0[b] - rg0[b0])
                    K = int(colK[b])
                    nc.vector.tensor_tensor(
                        out=araw[:, r0:r0 + K, :], in0=g[:, r0:r0 + K, 0:HID:C],
                        in1=sdst_all[:, b:b + 1, :].broadcast_to([128, K, H]),
                        op=ALU.add)
                # chunk-wide: slat add, leaky relu
                nc.vector.tensor_tensor(out=araw[:, 0:nct, :], in0=araw[:, 0:nct, :],
                                        in1=cst['slat'][:, cb:cb + nct, :], op=ALU.add)
                nc.vector.scalar_tensor_tensor(out=araw[:, 0:nct, :],
                                               in0=araw[:, 0:nct, :], scalar=0.2,
                                               in1=araw[:, 0:nct, :],
                                               op0=ALU.mult, op1=ALU.max)
                # per-block: amax, sub
                amax = slabp.tile([128, cn, H], F32, tag="amax")
                for bi in range(cn):
                    b = b0 + bi
                    r0 = int(rg0[b] - rg0[b0])
                    K = int(colK[b])
                    nc.vector.tensor_reduce(
                        out=amax[:, bi, :],
                        in_=araw[:, r0:r0 + K, :].rearrange("p k h -> p h k"),
                        op=ALU.max, axis=AX.X)
                    nc.vector.tensor_tensor(
                        out=araw[:, r0:r0 + K, :], in0=araw[:, r0:r0 + K, :],
                        in1=amax[:, bi:bi + 1, :].broadcast_to([128, K, H]),
                        op=ALU.subtract)
                wexp = slabp.tile([128, MAXCOL, H], BF16, tag="wexp")
                nc.scalar.activation(out=wexp[:, 0:nct, :], in_=araw[:, 0:nct, :],
                                     func=ACTF.Exp)
                den = slabp.tile([128, cn, H], F32, tag="den")
                for bi in range(cn):
                    b = b0 + bi
                    r0 = int(rg0[b] - rg0[b0])
                    K = int(colK[b])
                    nc.vector.tensor_reduce(
                        out=den[:, bi, :],
                        in_=wexp[:, r0:r0 + K, :].rearrange("p k h -> p h k"),
                        op=ALU.add, axis=AX.X)
                nc.vector.tensor_scalar(out=den[:], in0=den[:], scalar1=1e-16,
                                        scalar2=None, op0=ALU.add)
                recip = slabp.tile([128, cn, H], F32, tag="recip")
                nc.vector.reciprocal(out=recip[:], in_=den[:])
                alpha = slabp.tile([128, MAXCOL, H], BF16, tag="alpha")
                for bi in range(cn):
                    b = b0 + bi
                    r0 = int(rg0[b] - rg0[b0])
                    K = int(colK[b])
                    nc.vector.tensor_tensor(
                        out=alpha[:, r0:r0 + K, :], in0=wexp[:, r0:r0 + K, :],
                        in1=recip[:, bi:bi + 1, :].broadcast_to([128, K, H]),
                        op=ALU.mult)
                wm = wmbufp.tile([128, MAXCOL, HID], BF16, tag="wm")
                nc.vector.tensor_tensor(
                    out=wm[:, 0:nct, :].rearrange("p n (h c) -> p n h c", c=C),
                    in0=g[:, 0:nct, :].rearrange("p n (h c) -> p n h c", c=C),
                    in1=alpha[:, 0:nct, :].rearrange("p n h -> p n h ()").broadcast_to([128, nct, H, C]),
                    op=ALU.mult)

                # ---- scatter + node phase per block ----
                for bi in range(cn):
                    b = b0 + bi
                    r0 = int(rg0[b] - rg0[b0])
                    K = int(colK[b])
                    ps = psS.tile([128, 128], F32, tag="scat")
                    for j in range(K):
                        nc.tensor.matmul(ps[:], wm[:, r0 + j, :], cst['ident_bf'][:],
                                         start=(j == 0), stop=(j == K - 1))
                    opsT = workp.tile([128, 128], BF16, tag="opsT")
                    nc.scalar.copy(out=opsT[:], in_=ps[:])
                    if not mlp:
                        ynT = psN.tile([128, 128], F32, tag="ynT")
                        nc.tensor.matmul(ynT[:], cst['QT'][:], opsT[:],
                                         start=True, stop=True)
                        xnT = workp.tile([128, 128], BF16, tag="xnT")
                        nc.vector.tensor_scalar(out=xnT[:], in0=ynT[:],
                                                scalar1=cst['b_col'][:, 0:1],
                                                scalar2=0.0, op0=ALU.add, op1=ALU.max)
                        xpp = psN.tile([128, 128], F32, tag="xpp")
                        nc.tensor.matmul(xpp[:], xnT[:], cst['Wq'][:],
                                         start=True, stop=True)
                        nc.scalar.copy(out=xpn[:, b, :], in_=xpp[:])
                    else:
                        h1 = []
                        for jh in range(2):
                            hp = psN.tile([128, 128], F32, tag="mlp1")
                            nc.tensor.matmul(hp[:],
                                             cst['QC1'][:, jh * 128:(jh + 1) * 128],
                                             opsT[:], start=True, stop=True)
                            hs = workp.tile([128, 128], BF16, tag=f"h1_{jh}")
                            nc.vector.tensor_scalar(out=hs[:], in0=hp[:],
                                                    scalar1=cst['c1b_col'][:, jh:jh + 1],
                                                    scalar2=0.0, op0=ALU.add, op1=ALU.max)
                            h1.append(hs)
                        h2 = []
                        for jh in range(2):
                            hp = psN.tile([128, 128], F32, tag="mlp2")
                            for kc in range(2):
                                nc.tensor.matmul(
                                    hp[:],
                                    cst['C2w'][:, (2 * kc + jh) * 128:(2 * kc + jh + 1) * 128],
                                    h1[kc][:], start=(kc == 0), stop=(kc == 1))
                            hs = workp.tile([128, 128], BF16, tag=f"h2_{jh}")
                            nc.vector.tensor_scalar(out=hs[:], in0=hp[:],
                                                    scalar1=cst['c2b_col'][:, jh:jh + 1],
                                                    scalar2=0.0, op0=ALU.add, op1=ALU.max)
                            h2.append(hs)
                        nvp = psN.tile([128, 1], F32, tag="nvp")
                        for kc in range(2):
                            nc.tensor.matmul(nvp[:], h2[kc][:], cst['C3w'][:, kc:kc + 1],
                                             start=(kc == 0), stop=(kc == 1))
                        nv = workp.tile([128, 1], F32, tag="nv")
                        nc.vector.tensor_scalar(out=nv[:], in0=nvp[:], scalar1=c3b,
                                                scalar2=0.0, op0=ALU.add, op1=ALU.max)
                        nc.tensor.matmul(gp[:], cst['pool_mat'][:, b * NG:(b + 1) * NG],
                                         nv[:], start=(b == 0), stop=(b == BPC - 1))

            if mlp:
                pt = workp.tile([NG, 1], F32, tag="pt")
                nc.scalar.copy(out=pt[:], in_=gp[:])
                nc.sync.dma_start(tout['partials'].ap(), pt[:])
            else:
                nc.sync.dma_start(
                    tout['xp_next'].ap().rearrange("(b p) j -> p b j", p=128), xpn[:])
    nc.compile()
    return nc


def _build_feat(host):
    import concourse.bacc as bacc
    import concourse.mybir as mybir
    import concourse.tile as tile
    from concourse import library_config
    F32 = mybir.dt.float32
    BF16 = mybir.dt.bfloat16
    ALU = mybir.AluOpType
    AX = mybir.AxisListType
    ACTF = mybir.ActivationFunctionType
    nc = bacc.Bacc("TRN2", target_bir_lowering=False, debug=False,
                   num_devices=NCORES)
    NB = NPAD // 128
    ins = {
        'req_w': ([128, NB], F32), 'us_own': ([128, BPC], F32),
        'mask_ge15': ([128, NB], F32), 'mask_lt15': ([128, NB], F32),
        'onehot4T': ([4, NPC], F32), 'T0q': ([4, HID], F32),
        'w16q_rep': ([128, HID], F32), 'w17q_rep': ([128, HID], F32),
        'ones_col': ([128, 1], F32),
    }
    tin = _mk(ins, nc, "ExternalInput")
    tout = _mk({'xp_next': ([NPC, HID], BF16)}, nc, "ExternalOutput")
    n = float(N - NL)
    with tile.TileContext(nc) as tc:
        with (
            tc.tile_pool(name="const", bufs=1) as constp,
            tc.tile_pool(name="work", bufs=3) as workp,
            tc.tile_pool(name="slice", bufs=1) as slicep,
            tc.tile_pool(name="ps", bufs=2, space="PSUM") as ps,
        ):
            nc.gpsimd.load_library(library_config.mlp)
            c = {}
            for name in ins:
                shape, dt = ins[name]
                t = constp.tile(list(shape), dt, tag=name)
                nc.sync.dma_start(t[:], tin[name].ap())
                c[name] = t
            d = workp.tile([128, NB], F32, tag="d")
            nc.vector.tensor_tensor(out=d[:], in0=c['req_w'][:], in1=c['mask_ge15'][:], op=ALU.mult)
            col = workp.tile([128, 1], F32, tag="col")
            nc.vector.tensor_reduce(out=col[:], in_=d[:], op=ALU.add, axis=AX.X)
            tot = ps.tile([1, 1], F32, tag="tot")
            nc.tensor.matmul(tot[:], col[:], c['ones_col'][:], start=True, stop=True)
            mean = workp.tile([1, 1], F32, tag="mean")
            nc.vector.tensor_scalar(out=mean[:], in0=tot[:], scalar1=1.0 / n, scalar2=None, op0=ALU.mult)
            mean_col = workp.tile([128, 1], F32, tag="mean_col")
            nc.gpsimd.partition_broadcast(mean_col[:], mean[:])
            nc.vector.tensor_scalar(out=d[:], in0=c['req_w'][:], scalar1=mean_col[:, 0:1], scalar2=None, op0=ALU.subtract)
            nc.vector.tensor_tensor(out=d[:], in0=d[:], in1=c['mask_ge15'][:], op=ALU.mult)
            d2 = workp.tile([128, NB], F32, tag="d2")
            nc.vector.tensor_tensor(out=d2[:], in0=d[:], in1=d[:], op=ALU.mult)
            nc.vector.tensor_reduce(out=col[:], in_=d2[:], op=ALU.add, axis=AX.X)
            tot2 = ps.tile([1, 1], F32, tag="tot2")
            nc.tensor.matmul(tot2[:], col[:], c['ones_col'][:], start=True, stop=True)
            var = workp.tile([1, 1], F32, tag="var")
            nc.vector.tensor_scalar(out=var[:], in0=tot2[:], scalar1=1.0 / (n - 1.0), scalar2=None, op0=ALU.mult)
            std = workp.tile([1, 1], F32, tag="std")
            nc.scalar.activation(out=std[:], in_=var[:], func=ACTF.Sqrt)
            nc.vector.tensor_scalar(out=std[:], in0=std[:], scalar1=1e-6, scalar2=None, op0=ALU.add)
            rinv = workp.tile([1, 1], F32, tag="rinv")
            nc.vector.reciprocal(out=rinv[:], in_=std[:])
            rinv_col = workp.tile([128, 1], F32, tag="rinv_col")
            nc.gpsimd.partition_broadcast(rinv_col[:], rinv[:])
            rf = workp.tile([128, NB], F32, tag="rf")
            nc.vector.tensor_scalar(out=rf[:], in0=d[:], scalar1=rinv_col[:, 0:1], scalar2=None, op0=ALU.mult)
            raw15 = workp.tile([128, NB], F32, tag="raw15")
            nc.vector.tensor_tensor(out=raw15[:], in0=c['req_w'][:], in1=c['mask_lt15'][:], op=ALU.mult)
            nc.vector.tensor_tensor(out=rf[:], in0=rf[:], in1=raw15[:], op=ALU.add)

            xpn = slicep.tile([128, BPC, HID], BF16, tag="xpn")
            for b in range(BPC):
                mm = ps.tile([128, HID], F32, tag="mm")
                nc.tensor.matmul(mm[:], c['onehot4T'][:, b * 128:(b + 1) * 128], c['T0q'][:],
                                 start=True, stop=True)
                x0 = workp.tile([128, HID], F32, tag="x0")
                nc.scalar.copy(out=x0[:], in_=mm[:])
                t1 = workp.tile([128, HID], F32, tag="t1")
                nc.vector.tensor_scalar(out=t1[:], in0=c['w16q_rep'][:], scalar1=rf[:, b:b + 1], scalar2=None, op0=ALU.mult)
                nc.vector.tensor_tensor(out=x0[:], in0=x0[:], in1=t1[:], op=ALU.add)
                nc.vector.tensor_scalar(out=t1[:], in0=c['w17q_rep'][:], scalar1=c['us_own'][:, b:b + 1], scalar2=None, op0=ALU.mult)
                nc.vector.tensor_tensor(out=xpn[:, b, :], in0=x0[:], in1=t1[:], op=ALU.add)
            nc.sync.dma_start(tout['xp_next'].ap().rearrange("(b p) j -> p b j", p=128), xpn[:])
    nc.compile()
    return nc


def _run(nc, in_maps, want_time=False):
    from concourse.bass_utils import run_bass_kernel_spmd
    t0 = time.monotonic()
    res = run_bass_kernel_spmd(nc, in_maps, core_ids=list(range(NCORES)))
    wall = (time.monotonic() - t0) * 1e9
    t = res.exec_time_ns if res.exec_time_ns else None
    _run.last_results.append(res)
    return res.results, (t if t else wall)


_run.last_results = []


def _assemble_tab(xp_slices):
    tab = np.zeros((2 * HROWS, HID), BF)
    tab[0:HALF] = np.concatenate(xp_slices[:4], axis=0)
    tab[HROWS:HROWS + HALF] = np.concatenate(xp_slices[4:], axis=0)
    return tab


def kernel(**inputs):
    key = 'k'
    if key not in _cache:
        host = _build_host({k: np.asarray(v) for k, v in inputs.items()})
        _cache[key] = (host, _build_feat(host), _build_gat(host, mlp=False),
                       _build_gat(host, mlp=True))
    host, p_feat, p_gat, p_mlp = _cache[key]
    times = []
    _run.last_results = []

    in_maps = []
    for k in range(NCORES):
        own = np.arange(k * BPC, (k + 1) * BPC)
        rest = np.array([cc for cc in range(NPAD // 128)
                         if not (k * BPC <= cc < (k + 1) * BPC)])
        permc = np.concatenate([own, rest])
        in_maps.append(dict(
            req_w=np.ascontiguousarray(host['req_w_full'][:, permc]),
            us_own=np.ascontiguousarray(host['us_w_full'][:, k * BPC:(k + 1) * BPC]),
            mask_ge15=np.ascontiguousarray(host['mask_ge15'][:, permc]),
            mask_lt15=np.ascontiguousarray(host['mask_lt15'][:, permc]),
            onehot4T=host['onehot4T'][k], T0q=host['T0q'],
            w16q_rep=host['w16q_rep'], w17q_rep=host['w17q_rep'],
            ones_col=host['ones_col']))
    res, t = _run(p_feat, in_maps)
    times.append(t)
    slices = [res[k]['xp_next'] for k in range(NCORES)]

    for li in range(4):
        L = host['layers'][li]
        mlp = (li == 3)
        tab = _assemble_tab(slices)
        in_maps = []
        for k in range(NCORES):
            rb = k * NPC + (ZPAD if k >= 4 else 0)
            m = dict(tab=tab, xp_own=np.ascontiguousarray(tab[rb:rb + NPC]),
                     idx_lo=host['idx_lo16'][k], idx_hi=host['idx_hi16'][k],
                     slat=L['slat'][k],
                     c0p_rep=L['c0p_rep'], c1p_rep=L['c1p_rep'],
                     ident_bf=host['ident_bf'])
            if mlp:
                M = host['mlp']
                m.update(QC1=M['QC1'], C2w=M['C2w'], C3w=M['C3w'],
                         c1b_col=M['c1b_col'], c2b_col=M['c2b_col'],
                         pool_mat=host['pool_mat'][k])
            else:
                m.update(QT=L['QT'], Wq=L['Wq'], b_col=L['b_col'])
            in_maps.append(m)
        res, t = _run(p_mlp if mlp else p_gat, in_maps)
        times.append(t)
        if not mlp:
            slices = [res[k]['xp_next'] for k in range(NCORES)]

    partials = sum(np.asarray(res[k]['partials'], np.float64) for k in range(NCORES))
    out = (partials[:, 0] / np.maximum(host['cnt'], 1.0)).astype(np.float32)[:, None]
    kernel._last_times = times
    return out
